# revision 34
# baseline (speedup 1.0000x reference)
"""Trainium2 Bass kernel for nn_Block_27187142983954 (dense transformer block,
per-position head-mixing attention). Data-parallel over batch: 8 cores, one
batch element each. Self-contained: hardcodes all shapes.

Per-core plan (S=4096 positions, E=1024, H=16 heads, D=64):
  - qkv projection on TensorE in fp8(e4m3) DoubleRow perf mode (K=256 per
    matmul): stationary = x feature-major fp8 tiles (host-pretransposed),
    moving = host-pretransposed fp8 weights prescaled x32 (restored at the
    PSUM->SBUF eviction); biases folded in as rank-1 (K=1) bf16 matmuls.
  - attention (per-position bilinear over heads) on VectorE in position-major
    layout with broadcast access patterns: bf16 tensor_tensor muls in 2x mode,
    partial reduction by halving-tree TT adds (2x) + final tensor_reduce.
  - softmax without max-subtraction (scores are O(1) by construction); the
    1/denominator is folded into exp(scores) BEFORE attn@v so no fp32
    broadcast-normalize is needed afterwards.
  - v is computed with host-permuted weight rows so its features land in
    (d,g) order, which keeps every broadcast AP's innermost dim contiguous.
  - proj/ff matmuls on TensorE with PE-transposed activations as stationary;
    the attn residual (x, bf16) is accumulated into the proj PSUM via an
    identity matmul so z1 needs no VectorE PSUM-read add.
  - LayerNorm stats on ScalarE via activation accum_out (Identity/Square);
    rsigma = exp(-0.5*ln(var+eps)) so softmax-exp and LN share one ACT table
    set; ln_g/ln_b of LN1 are folded into the ff weights on the host; LN2's
    affine is skipped entirely when ln_g==1 and ln_b==0 (program variant).
  - engine placement discipline: keeping ScalarE/GpSimd co-activity low
    matters more than offloading VectorE — heavy co-activity inflates every
    engine's per-op time ~20% (SBUF contention / power throttle).
"""

import sys

sys.path.insert(0, "/opt/trn_rl_repo")

import numpy as np
import ml_dtypes

E, H, DQ, DV = 1024, 16, 64, 64
B, S = 8, 4096
EPS = 1e-5
NT = S // 128  # 32 position tiles per core
BF = ml_dtypes.bfloat16
F8 = ml_dtypes.float8_e4m3
W8SCALE = 32.0  # qkv weights are ~1/32; prescale into fp8's normal range

_CACHE = {}


def _patch_tail_drain():
    """walrus in this container rejects >1 sem wait on a CTRL (Drain)
    instruction; spread the TileContext tail-drain waits over wait-nops."""
    import concourse.tile as tile
    import bass_rust
    from concourse.vector_clock import ScopedClock

    if getattr(tile.TileContext, "_drain_patched", False):
        return

    def _drain_and_barrier(self, tick_clock, wait_clock):
        nc = self.nc
        drain_inst = nc.sync.drain()
        wait_clock.add_sem_waits(
            drain_inst.ins, ScopedClock({None: tick_clock.global_clock})
        )
        si = drain_inst.ins.sync_info
        waits = list(si.on_wait) if si is not None else []
        if len(waits) > 1:
            drain_inst.ins.sync_info = bass_rust.SyncInfo(on_wait=[], on_update=[])
            for w in waits:
                nop = nc.sync.nop()
                nop.ins.sync_info = bass_rust.SyncInfo(on_wait=[w], on_update=[])
        nc.all_engine_barrier()
        assert self.sems is not None
        popped = nc._tile_sem_poison_stack.pop()
        assert popped is self._sem_poison
        nc.clear_and_free_semaphores(list(self.sems.allocated().values()))
        nc.all_engine_barrier()

    tile.TileContext._drain_and_barrier = _drain_and_barrier
    tile.TileContext._drain_patched = True


def _split_excess_waits(nc, max_on_op=1, max_on_nop=1):
    """walrus in this container rejects >2 sem waits on compute instruction
    structs and >1 on DMA/CTRL structs. Hoist excess waits onto preceding
    same-engine NOPs."""
    import concourse.mybir as mybir
    import bass_rust

    narrow = {"DMACopy", "Drain", "NoOp", "Memset", "TriggeredCopy"}
    cnt = 0
    for bb in nc.m.functions[0].blocks:
        il = bb.instructions
        out = []
        for inst in il:
            cap = 1 if inst.opcode in narrow else max_on_op
            si = inst.sync_info
            waits = list(si.on_wait) if si is not None and si.on_wait else []
            if len(waits) > cap:
                n_extra = len(waits) - cap
                extra, keep = waits[:n_extra], waits[n_extra:]
                for i0 in range(0, len(extra), max_on_nop):
                    chunk = extra[i0 : i0 + max_on_nop]
                    nop = mybir.InstNoOp(name=f"waitnop-{cnt}", ins=[], outs=[])
                    cnt += 1
                    nop.engine = inst.engine
                    nop.sync_info = bass_rust.SyncInfo(on_wait=chunk, on_update=[])
                    out.append(nop)
                inst.sync_info = bass_rust.SyncInfo(
                    on_wait=keep,
                    on_update=list(si.on_update) if si.on_update else [],
                )
            out.append(inst)
        il[:] = out


def _build_program(trivial_affine: bool, variant: int = 0):
    import concourse.bass as bass
    import concourse.tile as tile
    import concourse.mybir as mybir
    from concourse.masks import make_identity

    _patch_tail_drain()

    f32 = mybir.dt.float32
    bf16 = mybir.dt.bfloat16
    fp8 = mybir.dt.float8e4
    ALU = mybir.AluOpType
    ACT = mybir.ActivationFunctionType
    DR = mybir.MatmulPerfMode.DoubleRow

    nc = bass.Bass("TRN2", target_bir_lowering=False, debug=False, num_devices=1)

    if variant < 7:
        x_pm = nc.dram_tensor("x_pm", [S, E], f32, kind="ExternalInput").ap()
    x_bf_d = nc.dram_tensor("x_bf", [S, E], bf16, kind="ExternalInput").ap()
    if variant >= 2:
        xsum_d = nc.dram_tensor("xsum", [S, 1], f32, kind="ExternalInput").ap()
    xT = nc.dram_tensor("xT", [E, S], fp8, kind="ExternalInput").ap()
    wqkvT_d = nc.dram_tensor("wqkvT", [E, 3 * E], fp8, kind="ExternalInput").ap()
    projT_d = nc.dram_tensor("projT", [E, E], bf16, kind="ExternalInput").ap()
    ffw2T_d = nc.dram_tensor("ffw2T", [E, E], bf16, kind="ExternalInput").ap()
    bqkv_d = nc.dram_tensor("bqkv", [1, 3 * E], bf16, kind="ExternalInput").ap()
    bproj_d = nc.dram_tensor("bproj", [1, E], bf16, kind="ExternalInput").ap()
    bff2_d = nc.dram_tensor("bff2", [1, E], bf16, kind="ExternalInput").ap()
    if not trivial_affine:
        g_rep_d = nc.dram_tensor("g_rep", [128, E], f32, kind="ExternalInput").ap()
        b_rep_d = nc.dram_tensor("b_rep", [128, E], f32, kind="ExternalInput").ap()
    out_d = nc.dram_tensor("out", [S, E], f32, kind="ExternalOutput").ap()

    xT_r = xT.rearrange("(t p) s -> p t s", p=128)  # [128, 8, S]
    wqkv_r = wqkvT_d.rearrange("(t p) o -> p t o", p=128)
    proj_r = projT_d.rearrange("(t p) o -> p t o", p=128)
    ffw2_r = ffw2T_d.rearrange("(t p) o -> p t o", p=128)

    with tile.TileContext(nc) as tc:
        import contextlib

        ctx = contextlib.ExitStack()
        with ctx:
            fixed = ctx.enter_context(tc.tile_pool(name="fixed", bufs=1))
            work = ctx.enter_context(
                tc.tile_pool(name="work", bufs=(4 if variant >= 6 else 3))
            )
            work1 = ctx.enter_context(tc.tile_pool(name="work1", bufs=1))
            if variant >= 10:
                # qkv double-buffered: the qkv stage runs one tile ahead so
                # the next tile's first prod never waits on ScalarE evictions
                qkvp = ctx.enter_context(tc.tile_pool(name="qkvp", bufs=2))
            stats = ctx.enter_context(tc.tile_pool(name="stats", bufs=8))
            psq = ctx.enter_context(
                tc.tile_pool(name="psq", bufs=(2 if variant >= 9 else 3), space="PSUM")
            )
            pst = ctx.enter_context(tc.tile_pool(name="pst", bufs=2, space="PSUM"))
            if variant >= 9:
                # 2-bank [128,1024] f32 tiles: proj and ff each evict in ONE
                # ScalarE pass (one accum read) instead of two + a pair-add
                psb = ctx.enter_context(tc.tile_pool(name="psz", bufs=2, space="PSUM"))
            else:
                psb = ctx.enter_context(
                    tc.tile_pool(
                        name="psb", bufs=(3 if variant >= 4 else 2), space="PSUM"
                    )
                )

            # ---- fixed tensors ----
            # v7: tile 0's x loads are hoisted ahead of the weight loads so
            # compute opens ~1.5MB into the DMA stream instead of ~7MB.
            wdma = nc.sync.dma_start
            pre = {}
            n_pre = 2 if variant >= 9 else 1
            if variant >= 7:
                # first tiles' inputs first: compute can start after ~0.7MB
                for pt_ in range(n_pre):
                    o = pt_ * 128
                    pre_xbf = work.tile([128, E], bf16, tag="xbf")
                    nc.sync.dma_start(out=pre_xbf, in_=x_bf_d[o : o + 128, :])
                    pre_xs = work.tile([128, 1], f32, tag="xs")
                    nc.sync.dma_start(out=pre_xs, in_=xsum_d[o : o + 128, :])
                    pre_xf = work.tile([128, 8, 128], fp8, tag="xf")
                    nc.sync.dma_start(out=pre_xf, in_=xT_r[:, :, o : o + 128])
                    pre[pt_] = (pre_xbf, pre_xs, pre_xf)
            bqkv_sb = fixed.tile([1, 3 * E], bf16)
            wdma(out=bqkv_sb, in_=bqkv_d)
            wqkv_sb = fixed.tile([128, 8, 3 * E], fp8)
            if variant:
                # column-group order: tile 0's first psum chunk only waits on
                # the first 1/6th of the weight load
                for j in range(6):
                    wdma(
                        out=wqkv_sb[:, :, j * 512 : (j + 1) * 512],
                        in_=wqkv_r[:, :, j * 512 : (j + 1) * 512],
                    )
            else:
                for t in range(8):
                    wdma(out=wqkv_sb[:, t, :], in_=wqkv_r[:, t, :])
            # v10: proj/ff weights ride the (otherwise idle) GpSimd SWDGE
            # queue so ~18 trigger slots don't serialize the sync queue at
            # startup in front of the per-tile x loads
            w2dma = nc.gpsimd.dma_start if variant >= 10 else wdma
            proj_sb = fixed.tile([128, 8, E], bf16)
            ffw2_sb = fixed.tile([128, 8, E], bf16)
            for t in range(8):
                w2dma(out=proj_sb[:, t, :], in_=proj_r[:, t, :])
            for t in range(8):
                w2dma(out=ffw2_sb[:, t, :], in_=ffw2_r[:, t, :])
            bproj_sb = fixed.tile([1, E], bf16)
            w2dma(out=bproj_sb, in_=bproj_d)
            bff2_sb = fixed.tile([1, E], bf16)
            w2dma(out=bff2_sb, in_=bff2_d)
            if not trivial_affine:
                g_rep = fixed.tile([128, E], f32)
                nc.sync.dma_start(out=g_rep, in_=g_rep_d)
                b_rep = fixed.tile([128, E], f32)
                nc.sync.dma_start(out=b_rep, in_=b_rep_d)
            ones_row = fixed.tile([1, 128], bf16)
            nc.vector.memset(ones_row, 1.0)
            ident = fixed.tile([128, 128], bf16)
            make_identity(nc, ident)
            eps_sb = fixed.tile([128, 1], f32)
            nc.vector.memset(eps_sb, EPS)

            inv_n = 1.0 / float(E)

            def emit_tail(gl_p, xp_p, s0_p, s1z2=None):
                """Deferred tile tail: z2 = gelu_out + x, LN2, affine, store.
                Emitted one iteration late so the DVE z2/LN ops land in the
                next tile's exp-wait window instead of stalling on gelu."""
                if variant >= 9:
                    # gl_p is the ff PSUM [128,1024] (2 banks); the gelu
                    # eviction itself is deferred to here so it lands right
                    # after the next tile's softmax exp in ScalarE's FIFO
                    # (the gelu ACT-table swap then sits off-critical too).
                    # s1z2 arrives as the xs tile: sum(z2) = sum(gelu) + sum(x).
                    gl = work.tile([128, E], bf16, tag="gl")
                    sgl = stats.tile([128, 1], f32, tag="sgl")
                    nc.scalar.activation(gl, gl_p, ACT.Gelu, accum_out=sgl)
                    s1t = stats.tile([128, 1], f32, tag="s1z2")
                    nc.scalar.activation(s1t, sgl, ACT.Identity, bias=s1z2)
                    s1z2 = s1t
                    gl_p = gl
                if variant >= 7:
                    # bf16 residual add: 2x DVE mode, and the f32 x load is
                    # dropped entirely (xp_p is the bf16 x tile here)
                    z2 = work.tile([128, E], bf16, tag="z2")
                    nc.vector.tensor_tensor(z2, gl_p, xp_p, ALU.add)
                else:
                    z2 = work.tile([128, E], f32, tag="z2")
                    for j in range(2):
                        nc.vector.tensor_tensor(
                            z2[:, j * 512 : (j + 1) * 512],
                            gl_p[:, j * 512 : (j + 1) * 512],
                            xp_p[:, j * 512 : (j + 1) * 512],
                            ALU.add,
                        )
                rs2 = stats.tile([128, 1], f32, tag="rs2")
                mrs2 = stats.tile([128, 1], f32, tag="mrs2")
                lnscr2 = work1.tile([128, E], bf16, tag="lnscr2")
                layer_norm(z2, rs2, mrs2, lnscr2, s1_pre=s1z2)
                if trivial_affine:
                    out_t = work.tile([128, E], f32, tag="out_t")
                    nc.scalar.activation(
                        out_t, z2, ACT.Identity, bias=mrs2, scale=rs2
                    )
                else:
                    zn = work1.tile([128, E], f32, tag="zn")
                    nc.scalar.activation(zn, z2, ACT.Identity, bias=mrs2, scale=rs2)
                    zn2 = work1.tile([128, E], f32, tag="zn2")
                    nc.gpsimd.tensor_tensor(zn2, zn, g_rep, ALU.mult)
                    out_t = work.tile([128, E], f32, tag="out_t")
                    nc.gpsimd.tensor_tensor(out_t, zn2, b_rep, ALU.add)
                nc.sync.dma_start(out=out_d[s0_p : s0_p + 128, :], in_=out_t)

            pending = None

            def layer_norm_scalar(s1, s2, rs_out, mrs_out):
                """LN scalar chain entirely on ScalarE: keeps every [P,1] op
                out of DVE's in-order queue so DVE never head-of-line blocks
                on cross-engine stats. rsigma = exp(-0.5*ln(s2/N + eps - mu^2))."""
                mneg = stats.tile([128, 1], f32, tag="mneg")
                nc.scalar.activation(mneg, s1, ACT.Identity, scale=-inv_n)
                mu2 = stats.tile([128, 1], f32, tag="mu2")
                nc.scalar.activation(mu2, s1, ACT.Square, scale=inv_n)
                emm = stats.tile([128, 1], f32, tag="emm")
                nc.scalar.activation(emm, mu2, ACT.Identity, scale=-1.0, bias=eps_sb)
                lnv = stats.tile([128, 1], f32, tag="lnv")
                nc.scalar.activation(lnv, s2, ACT.Ln, scale=inv_n, bias=emm)
                nc.scalar.activation(rs_out, lnv, ACT.Exp, scale=-0.5)
                nc.scalar.activation(mrs_out, mneg, ACT.Identity, scale=rs_out)

            def layer_norm(z, rs_out, mrs_out, scratch_bf, s1_pre=None, s2_pre=None):
                """Compute rsigma and -mu*rsigma of z [128, E] (fp32).
                If s1_pre/s2_pre are given (sum / sum-of-squares precomputed,
                e.g. fused into the PSUM eviction), those passes are skipped."""
                if s1_pre is None:
                    s1 = stats.tile([128, 1], f32, tag="s1")
                    nc.scalar.activation(scratch_bf, z, ACT.Identity, accum_out=s1)
                else:
                    s1 = s1_pre
                if s2_pre is None:
                    s2 = stats.tile([128, 1], f32, tag="s2")
                    nc.scalar.activation(scratch_bf, z, ACT.Square, accum_out=s2)
                else:
                    s2 = s2_pre
                if variant >= 8:
                    layer_norm_scalar(s1, s2, rs_out, mrs_out)
                    return
                mu = stats.tile([128, 1], f32, tag="mu")
                lnv = stats.tile([128, 1], f32, tag="lnv")
                nc.vector.tensor_scalar_mul(mu, s1, inv_n)
                mu2 = stats.tile([128, 1], f32, tag="mu2")
                nc.vector.tensor_tensor(mu2, mu, mu, ALU.mult)
                var = stats.tile([128, 1], f32, tag="var")
                nc.vector.scalar_tensor_tensor(
                    var, in0=s2, scalar=inv_n, in1=mu2, op0=ALU.mult, op1=ALU.subtract
                )
                nc.scalar.activation(lnv, var, ACT.Ln, bias=eps_sb)
                nc.scalar.activation(rs_out, lnv, ACT.Exp, scale=-0.5)
                nc.vector.scalar_tensor_tensor(
                    mrs_out, in0=mu, scalar=-1.0, in1=rs_out, op0=ALU.mult, op1=ALU.mult
                )

            def qkv_stage(t):
                """x loads + qkv projection for tile t; returns (xbf, xs, qkv_sb).
                For variant>=10 this runs one tile AHEAD of the main body."""
                s0 = t * 128
                if t in pre:
                    xbf, xs, xf = pre[t]
                else:
                    xbf = work.tile([128, E], bf16, tag="xbf")
                    nc.sync.dma_start(out=xbf, in_=x_bf_d[s0 : s0 + 128, :])
                    xs = work.tile([128, 1], f32, tag="xs")
                    nc.sync.dma_start(out=xs, in_=xsum_d[s0 : s0 + 128, :])
                    xf = work.tile([128, 8, 128], fp8, tag="xf")
                    nc.sync.dma_start(out=xf, in_=xT_r[:, :, s0 : s0 + 128])
                qkv_sb = qkvp.tile([128, 3 * E], bf16, tag="qkv")
                for j in range(6):
                    ps = psq.tile([128, 512], f32, tag="psq")
                    for e2 in range(4):
                        nc.tensor.matmul(
                            ps,
                            xf[:, 2 * e2 : 2 * e2 + 2, :],
                            wqkv_sb[:, 2 * e2 : 2 * e2 + 2, j * 512 : (j + 1) * 512],
                            start=(e2 == 0),
                            stop=False,
                            perf_mode=DR,
                        )
                    nc.tensor.matmul(
                        ps,
                        ones_row,
                        bqkv_sb[:, j * 512 : (j + 1) * 512],
                        start=False,
                        stop=True,
                    )
                    nc.scalar.mul(qkv_sb[:, j * 512 : (j + 1) * 512], ps, 1.0 / W8SCALE)
                return xbf, xs, qkv_sb

            cur = None
            for t in range(NT):
                s0 = t * 128
                if variant >= 10:
                    xp = None
                    if t == 0:
                        cur = qkv_stage(0)
                    xbf, xs, qkv_sb = cur
                elif variant >= 7:
                    xp = None
                    if t in pre:
                        xbf, xs, xf = pre[t]
                    else:
                        xbf = work.tile([128, E], bf16, tag="xbf")
                        nc.sync.dma_start(out=xbf, in_=x_bf_d[s0 : s0 + 128, :])
                        xs = work.tile([128, 1], f32, tag="xs")
                        nc.sync.dma_start(out=xs, in_=xsum_d[s0 : s0 + 128, :])
                        xf = work.tile([128, 8, 128], fp8, tag="xf")
                        nc.sync.dma_start(out=xf, in_=xT_r[:, :, s0 : s0 + 128])
                else:
                    xp = work.tile([128, E], f32, tag="xp")
                    nc.sync.dma_start(out=xp, in_=x_pm[s0 : s0 + 128, :])
                    xbf = work.tile([128, E], bf16, tag="xbf")
                    nc.sync.dma_start(out=xbf, in_=x_bf_d[s0 : s0 + 128, :])
                    if variant >= 2:
                        xs = work.tile([128, 1], f32, tag="xs")
                        nc.sync.dma_start(out=xs, in_=xsum_d[s0 : s0 + 128, :])
                    xf = work.tile([128, 8, 128], fp8, tag="xf")
                    nc.sync.dma_start(out=xf, in_=xT_r[:, :, s0 : s0 + 128])

                if variant < 10:
                    # ---- qkv projection (fp8 DoubleRow, weights prescaled x32) ----
                    qkv_sb = work1.tile([128, 3 * E], bf16, tag="qkv")
                    for j in range(6):
                        ps = psq.tile([128, 512], f32, tag="psq")
                        for e2 in range(4):
                            nc.tensor.matmul(
                                ps,
                                xf[:, 2 * e2 : 2 * e2 + 2, :],
                                wqkv_sb[:, 2 * e2 : 2 * e2 + 2, j * 512 : (j + 1) * 512],
                                start=(e2 == 0),
                                stop=False,
                                perf_mode=DR,
                            )
                        nc.tensor.matmul(
                            ps,
                            ones_row,
                            bqkv_sb[:, j * 512 : (j + 1) * 512],
                            start=False,
                            stop=True,
                        )
                        nc.scalar.mul(
                            qkv_sb[:, j * 512 : (j + 1) * 512], ps, 1.0 / W8SCALE
                        )

                q3 = qkv_sb[:, 0:E].rearrange("p (h d) -> p h d", h=H)
                k3 = qkv_sb[:, E : 2 * E].rearrange("p (g d) -> p g d", g=H)
                v3 = qkv_sb[:, 2 * E : 3 * E].rearrange("p (d g) -> p d g", d=DV)

                # ---- QK^T scores ----
                prod = work1.tile([128, 8, 16, 64], bf16, tag="prod")
                scr = work1.tile([128, 8192], bf16, tag="scr")
                scores = work.tile([128, H, H], bf16 if variant >= 8 else f32, tag="scores")
                p_sb = work.tile([128, H, H], bf16, tag="p_sb")
                for half in range(2):
                    h0 = half * 8
                    qb = q3[:, h0 : h0 + 8, :].unsqueeze(2).broadcast_to([128, 8, 16, 64])
                    kb = k3.unsqueeze(1).broadcast_to([128, 8, 16, 64])
                    nc.vector.tensor_tensor(prod, kb, qb, ALU.mult)
                    t1 = scr[:, 0:4096].rearrange("p (a g d) -> p a g d", a=8, g=16)
                    nc.vector.tensor_tensor(
                        t1, prod[:, :, :, 0:32], prod[:, :, :, 32:64], ALU.add
                    )
                    t2 = scr[:, 4096:6144].rearrange("p (a g d) -> p a g d", a=8, g=16)
                    nc.vector.tensor_tensor(
                        t2, t1[:, :, :, 0:16], t1[:, :, :, 16:32], ALU.add
                    )
                    t3 = scr[:, 6144:7168].rearrange("p (a g d) -> p a g d", a=8, g=16)
                    nc.vector.tensor_tensor(
                        t3, t2[:, :, :, 0:8], t2[:, :, :, 8:16], ALU.add
                    )
                    t4 = scr[:, 7168:7680].rearrange("p (a g d) -> p a g d", a=8, g=16)
                    nc.vector.tensor_tensor(
                        t4, t3[:, :, :, 0:4], t3[:, :, :, 4:8], ALU.add
                    )
                    if variant >= 8:
                        # finish with 2x-mode TT halvings instead of a 1x reduce
                        t5 = scr[:, 7680:7936].rearrange(
                            "p (a g d) -> p a g d", a=8, g=16
                        )
                        nc.vector.tensor_tensor(
                            t5, t4[:, :, :, 0:2], t4[:, :, :, 2:4], ALU.add
                        )
                        nc.vector.tensor_tensor(
                            scores[:, h0 : h0 + 8, :].unsqueeze(3),
                            t5[:, :, :, 0:1],
                            t5[:, :, :, 1:2],
                            ALU.add,
                        )
                    else:
                        nc.vector.tensor_reduce(
                            scores[:, h0 : h0 + 8, :],
                            t4,
                            axis=mybir.AxisListType.X,
                            op=ALU.add,
                        )

                # ---- softmax (no max-subtraction; fold 1/den into p before AV) ----
                nc.scalar.activation(p_sb, scores, ACT.Exp)
                if pending is not None and variant < 9:
                    emit_tail(*pending)
                    pending = None
                den = stats.tile([128, H], f32, tag="den")
                nc.vector.tensor_reduce(
                    den, p_sb, axis=mybir.AxisListType.X, op=ALU.add
                )
                rden = stats.tile([128, H], f32, tag="rden")
                nc.vector.reciprocal(rden, den)
                p_nm = work.tile([128, H, H], bf16, tag="p_nm")
                nc.vector.tensor_tensor(
                    p_nm,
                    p_sb,
                    rden.unsqueeze(2).broadcast_to([128, H, H]),
                    ALU.mult,
                )
                if pending is not None:
                    # v9: tail emitted after den/rden/p_nm so DVE's in-order
                    # queue hits den (waits only on exp) before z2 (waits on
                    # the deferred gelu eviction)
                    emit_tail(*pending)
                    pending = None
                if variant >= 10 and t + 1 < NT:
                    # next tile's qkv stage here: its ScalarE evictions land
                    # ahead of this tile's LN/eviction stream, so the next
                    # tile's first prod has its inputs a full tile early
                    cur = qkv_stage(t + 1)

                # ---- attn @ v ----
                attn_bf = work.tile([128, E], bf16, tag="attn_bf")
                a3 = attn_bf.rearrange("p (h d) -> p h d", h=H)
                prod_flat = prod.rearrange("p a g d -> p (a g d)")
                for half in range(2):
                    h0 = half * 8
                    # reuse prod's memory with a contiguous [128, 8, 64, 16] layout
                    pa = prod_flat.rearrange("p (a d g) -> p a d g", a=8, d=DV)
                    pb = (
                        p_nm[:, h0 : h0 + 8, :]
                        .unsqueeze(2)
                        .broadcast_to([128, 8, 64, 16])
                    )
                    vb = v3.unsqueeze(1).broadcast_to([128, 8, 64, 16])
                    nc.vector.tensor_tensor(pa, vb, pb, ALU.mult)
                    u1 = scr[:, 0:4096].rearrange("p (a d g) -> p a d g", a=8, d=64)
                    nc.vector.tensor_tensor(
                        u1, pa[:, :, :, 0:8], pa[:, :, :, 8:16], ALU.add
                    )
                    u2 = scr[:, 4096:6144].rearrange("p (a d g) -> p a d g", a=8, d=64)
                    nc.vector.tensor_tensor(
                        u2, u1[:, :, :, 0:4], u1[:, :, :, 4:8], ALU.add
                    )
                    u3 = scr[:, 6144:7168].rearrange("p (a d g) -> p a d g", a=8, d=64)
                    nc.vector.tensor_tensor(
                        u3, u2[:, :, :, 0:2], u2[:, :, :, 2:4], ALU.add
                    )
                    nc.vector.tensor_tensor(
                        a3[:, h0 : h0 + 8, :].unsqueeze(3),
                        u3[:, :, :, 0:1],
                        u3[:, :, :, 1:2],
                        ALU.add,
                    )

                # ---- transpose attn_out to feature-major (4 or 8 per PSUM tile) ----
                # v7: PSUM->SBUF evictions on ScalarE (DVE is the bottleneck)
                evict = nc.scalar.copy if variant >= 7 else nc.vector.tensor_copy
                attn_fm = work.tile([128, 8, 128], bf16, tag="attn_fm")
                TG = 8 if variant >= 5 else 4
                for q in range(8 // TG):
                    pt = pst.tile([128, TG * 128], bf16, tag="pst")
                    for e4 in range(TG):
                        e = q * TG + e4
                        nc.tensor.transpose(
                            pt[:, e4 * 128 : (e4 + 1) * 128],
                            attn_bf[:, e * 128 : (e + 1) * 128],
                            ident,
                        )
                    evict(
                        attn_fm[:, q * TG : (q + 1) * TG, :].rearrange(
                            "p a b -> p (a b)"
                        ),
                        pt,
                    )

                # ---- proj + residual (x folded in via identity matmul) ----
                z1 = work1.tile([128, E], f32, tag="z1")
                lnscr = work1.tile([128, E], bf16, tag="lnscr")
                s1parts = []
                s2parts = []
                ps2w = None
                for j in range(2):
                    if variant >= 9:
                        if j == 0:
                            ps2w = psb.tile([128, 1024], f32, tag="psz")
                        ps2 = ps2w[:, j * 512 : (j + 1) * 512]
                    else:
                        ps2 = psb.tile([128, 512], f32, tag="psb")
                    for e in range(8):
                        nc.tensor.matmul(
                            ps2,
                            attn_fm[:, e, :],
                            proj_sb[:, e, j * 512 : (j + 1) * 512],
                            start=(e == 0),
                            stop=False,
                        )
                    nc.tensor.matmul(
                        ps2,
                        ones_row,
                        bproj_sb[:, j * 512 : (j + 1) * 512],
                        start=False,
                        stop=False,
                    )
                    nc.tensor.matmul(
                        ps2,
                        ident,
                        xbf[:, j * 512 : (j + 1) * 512],
                        start=False,
                        stop=True,
                    )
                    if variant >= 9:
                        continue
                    if variant:
                        # fuse the sum-of-z1 accumulation into the eviction
                        s1p = stats.tile([128, 1], f32, tag=f"s1p{j}")
                        s1parts.append(s1p)
                        nc.scalar.activation(
                            z1[:, j * 512 : (j + 1) * 512],
                            ps2,
                            ACT.Identity,
                            accum_out=s1p,
                        )
                        if variant >= 3:
                            # sum-of-squares per chunk straight from PSUM too
                            s2p = stats.tile([128, 1], f32, tag=f"s2p{j}")
                            s2parts.append(s2p)
                            nc.scalar.activation(
                                lnscr[:, j * 512 : (j + 1) * 512],
                                ps2,
                                ACT.Square,
                                accum_out=s2p,
                            )
                    else:
                        nc.scalar.copy(z1[:, j * 512 : (j + 1) * 512], ps2)

                # ---- LN1 (g,b folded into ff weights) ----
                rs1 = stats.tile([128, 1], f32, tag="rs1")
                mrs1 = stats.tile([128, 1], f32, tag="mrs1")
                if variant >= 9:
                    # single [128,1024] eviction across both PSUM banks: one
                    # accum read per stat, no pair-adds
                    s1p = stats.tile([128, 1], f32, tag="s1p")
                    nc.scalar.activation(z1, ps2w, ACT.Identity, accum_out=s1p)
                    s2p = stats.tile([128, 1], f32, tag="s2p")
                    nc.scalar.activation(lnscr, ps2w, ACT.Square, accum_out=s2p)
                    layer_norm(z1, rs1, mrs1, lnscr, s1_pre=s1p, s2_pre=s2p)
                elif variant:
                    s1f = stats.tile([128, 1], f32, tag="s1f")
                    if variant >= 7:
                        nc.scalar.activation(
                            s1f, s1parts[0], ACT.Identity, bias=s1parts[1]
                        )
                    else:
                        nc.vector.tensor_tensor(s1f, s1parts[0], s1parts[1], ALU.add)
                    if variant >= 3:
                        s2f = stats.tile([128, 1], f32, tag="s2f")
                        if variant >= 7:
                            nc.scalar.activation(
                                s2f, s2parts[0], ACT.Identity, bias=s2parts[1]
                            )
                        else:
                            nc.vector.tensor_tensor(
                                s2f, s2parts[0], s2parts[1], ALU.add
                            )
                        layer_norm(z1, rs1, mrs1, lnscr, s1_pre=s1f, s2_pre=s2f)
                    else:
                        layer_norm(z1, rs1, mrs1, lnscr, s1_pre=s1f)
                else:
                    layer_norm(z1, rs1, mrs1, lnscr)
                ln1_bf = work.tile([128, E], bf16, tag="ln1_bf")
                nc.scalar.activation(ln1_bf, z1, ACT.Identity, bias=mrs1, scale=rs1)

                ln1_fm = work.tile([128, 8, 128], bf16, tag="ln1_fm")
                for q in range(8 // TG):
                    pt = pst.tile([128, TG * 128], bf16, tag="pst")
                    for e4 in range(TG):
                        e = q * TG + e4
                        nc.tensor.transpose(
                            pt[:, e4 * 128 : (e4 + 1) * 128],
                            ln1_bf[:, e * 128 : (e + 1) * 128],
                            ident,
                        )
                    evict(
                        ln1_fm[:, q * TG : (q + 1) * TG, :].rearrange(
                            "p a b -> p (a b)"
                        ),
                        pt,
                    )

                # ---- ff + gelu (z2/LN2 deferred to next iteration's tail) ----
                if variant >= 9:
                    # matmuls only; the gelu eviction is deferred into the
                    # next iteration's tail (after its softmax exp)
                    ps3w = psb.tile([128, 1024], f32, tag="psz")
                    for j in range(2):
                        pj = ps3w[:, j * 512 : (j + 1) * 512]
                        for e in range(8):
                            nc.tensor.matmul(
                                pj,
                                ln1_fm[:, e, :],
                                ffw2_sb[:, e, j * 512 : (j + 1) * 512],
                                start=(e == 0),
                                stop=False,
                            )
                        nc.tensor.matmul(
                            pj,
                            ones_row,
                            bff2_sb[:, j * 512 : (j + 1) * 512],
                            start=False,
                            stop=True,
                        )
                    pending = (ps3w, xbf, s0, xs)
                    continue
                gl = work.tile([128, E], bf16 if variant >= 7 else f32, tag="gl")
                sglparts = []
                for j in range(2):
                    ps3 = psb.tile([128, 512], f32, tag="psb")
                    for e in range(8):
                        nc.tensor.matmul(
                            ps3,
                            ln1_fm[:, e, :],
                            ffw2_sb[:, e, j * 512 : (j + 1) * 512],
                            start=(e == 0),
                            stop=False,
                        )
                    nc.tensor.matmul(
                        ps3,
                        ones_row,
                        bff2_sb[:, j * 512 : (j + 1) * 512],
                        start=False,
                        stop=True,
                    )
                    if variant >= 2:
                        # fuse the sum-of-gelu accumulation into the eviction;
                        # sum(z2) = sum(gelu) + sum(x) (host-precomputed xsum)
                        sgl = stats.tile([128, 1], f32, tag=f"sgl{j}")
                        sglparts.append(sgl)
                        nc.scalar.activation(
                            gl[:, j * 512 : (j + 1) * 512],
                            ps3,
                            ACT.Gelu,
                            accum_out=sgl,
                        )
                    else:
                        nc.scalar.activation(
                            gl[:, j * 512 : (j + 1) * 512], ps3, ACT.Gelu
                        )

                if variant == 8:
                    # prefetch the exp/ln ACT table now (gelu swapped it out),
                    # so the next tile's softmax exp isn't stuck behind a
                    # 1.3us ACT_TABLE_LOAD on the critical path
                    atld = stats.tile([128, 1], f32, tag="atld")
                    nc.scalar.activation(atld, eps_sb, ACT.Ln)
                    atld2 = stats.tile([128, 1], f32, tag="atld2")
                    nc.scalar.activation(atld2, eps_sb, ACT.Exp)

                if variant >= 7:
                    # [P,1] pair-adds ride ScalarE (bias is added pre-func)
                    sgf = stats.tile([128, 1], f32, tag="sgf")
                    nc.scalar.activation(
                        sgf, sglparts[0], ACT.Identity, bias=sglparts[1]
                    )
                    s1z2 = stats.tile([128, 1], f32, tag="s1z2")
                    nc.scalar.activation(s1z2, sgf, ACT.Identity, bias=xs)
                    pending = (gl, xbf, s0, s1z2)
                elif variant >= 2:
                    sgf = stats.tile([128, 1], f32, tag="sgf")
                    nc.vector.tensor_tensor(sgf, sglparts[0], sglparts[1], ALU.add)
                    s1z2 = stats.tile([128, 1], f32, tag="s1z2")
                    nc.vector.tensor_tensor(s1z2, sgf, xs, ALU.add)
                    pending = (gl, xp, s0, s1z2)
                else:
                    pending = (gl, xp, s0)

            emit_tail(*pending)

    _split_excess_waits(nc)
    return nc


def _host_prep(inputs, trivial_affine=None):
    x = np.asarray(inputs["x"], np.float32)
    qk_w = np.asarray(inputs["qk_w"], np.float32)
    qk_b = np.asarray(inputs["qk_b"], np.float32)
    v_w = np.asarray(inputs["v_w"], np.float32)
    v_b = np.asarray(inputs["v_b"], np.float32)
    proj_w = np.asarray(inputs["proj_w"], np.float32)
    proj_b = np.asarray(inputs["proj_b"], np.float32)
    ff_w = np.asarray(inputs["ff_w"], np.float32)
    ff_b = np.asarray(inputs["ff_b"], np.float32)
    ln_g = np.asarray(inputs["ln_g"], np.float32)
    ln_b = np.asarray(inputs["ln_b"], np.float32)

    if trivial_affine is None:
        trivial_affine = bool(
            np.allclose(ln_g, 1.0, atol=1e-7) and np.allclose(ln_b, 0.0, atol=1e-7)
        )

    scale = 1.0 / np.sqrt(DQ).astype(np.float32)
    Wq = qk_w[:E] * scale
    bq = qk_b[:E] * scale
    Wk = qk_w[E:]
    bk = qk_b[E:]
    g_idx, d_idx = np.meshgrid(np.arange(H), np.arange(DV), indexing="ij")
    perm = np.empty(E, np.int64)
    perm[(d_idx * H + g_idx).ravel()] = (g_idx * DV + d_idx).ravel()
    Wv2 = v_w[perm]
    bv2 = v_b[perm]

    wqkvT = np.ascontiguousarray(
        (np.concatenate([Wq, Wk, Wv2], 0) * W8SCALE).T.astype(F8)
    )  # [E, 3E] fp8, prescaled
    bqkv = (np.concatenate([bq, bk, bv2]) * W8SCALE)[None, :].astype(BF)  # [1, 3E]
    projT = np.ascontiguousarray(proj_w.T.astype(BF))  # [E, E]
    bproj = proj_b[None, :].astype(BF)
    ffw2T = np.ascontiguousarray((ff_w * ln_g[None, :]).T.astype(BF))
    bff2 = (ff_b + ff_w @ ln_b)[None, :].astype(BF)

    shared = {
        "wqkvT": wqkvT,
        "bqkv": bqkv,
        "projT": projT,
        "bproj": bproj,
        "ffw2T": ffw2T,
        "bff2": bff2,
    }
    if not trivial_affine:
        shared["g_rep"] = np.ascontiguousarray(
            np.broadcast_to(ln_g[None, :], (128, E)), np.float32
        )
        shared["b_rep"] = np.ascontiguousarray(
            np.broadcast_to(ln_b[None, :], (128, E)), np.float32
        )
    in_maps = []
    for b in range(B):
        xb = np.ascontiguousarray(x[b])  # [S, E] f32
        xTb = np.ascontiguousarray(xb.T.astype(F8))  # [E, S] fp8
        m = {
            "x_pm": xb,
            "x_bf": xb.astype(BF),
            "xT": xTb,
            "xsum": np.ascontiguousarray(xb.sum(-1, dtype=np.float32)[:, None]),
        }
        m.update(shared)
        in_maps.append(m)
    return in_maps


def kernel(**inputs) -> np.ndarray:
    from concourse.bass_utils import run_bass_kernel_spmd

    trivial_affine = bool(
        np.allclose(np.asarray(inputs["ln_g"]), 1.0, atol=1e-7)
        and np.allclose(np.asarray(inputs["ln_b"]), 0.0, atol=1e-7)
    )
    variant = 10  # v9 + qkv stage pipelined one tile ahead, SWDGE weight loads
    key = ("nc", trivial_affine, variant)
    if key not in _CACHE:
        _CACHE[key] = _build_program(trivial_affine, variant)
    nc = _CACHE[key]

    in_maps = _host_prep(inputs, trivial_affine)
    res = run_bass_kernel_spmd(nc, in_maps, core_ids=list(range(B)))
    out = np.stack([res.results[b]["out"] for b in range(B)], 0)
    return out.astype(np.float32)


if __name__ == "__main__":
    rng = np.random.default_rng(0)
    ins = {
        "x": rng.standard_normal((B, S, E), np.float32),
        "qk_w": rng.standard_normal((2 * E, E), np.float32) * 0.03,
        "qk_b": rng.standard_normal((2 * E,), np.float32) * 0.03,
        "v_w": rng.standard_normal((E, E), np.float32) * 0.03,
        "v_b": rng.standard_normal((E,), np.float32) * 0.03,
        "proj_w": rng.standard_normal((E, E), np.float32) * 0.03,
        "proj_b": rng.standard_normal((E,), np.float32) * 0.03,
        "ff_w": rng.standard_normal((E, E), np.float32) * 0.03,
        "ff_b": rng.standard_normal((E,), np.float32) * 0.03,
        "ln_g": np.ones((E,), np.float32),
        "ln_b": np.zeros((E,), np.float32),
    }
    o = kernel(**ins)
    print("ran", o.shape, o.dtype)



# revision 35
# speedup vs baseline: 1.0062x; 1.0062x over previous
"""Trainium2 Bass kernel for nn_Block_27187142983954 (dense transformer block,
per-position head-mixing attention). Data-parallel over batch: 8 cores, one
batch element each. Self-contained: hardcodes all shapes.

Per-core plan (S=4096 positions, E=1024, H=16 heads, D=64):
  - qkv projection on TensorE in fp8(e4m3) DoubleRow perf mode (K=256 per
    matmul): stationary = x feature-major fp8 tiles (host-pretransposed),
    moving = host-pretransposed fp8 weights prescaled x32 (restored at the
    PSUM->SBUF eviction); biases folded in as rank-1 (K=1) bf16 matmuls.
  - attention (per-position bilinear over heads) on VectorE in position-major
    layout with broadcast access patterns: bf16 tensor_tensor muls in 2x mode,
    partial reduction by halving-tree TT adds (2x) + final tensor_reduce.
  - softmax without max-subtraction (scores are O(1) by construction); the
    1/denominator is folded into exp(scores) BEFORE attn@v so no fp32
    broadcast-normalize is needed afterwards.
  - v is computed with host-permuted weight rows so its features land in
    (d,g) order, which keeps every broadcast AP's innermost dim contiguous.
  - proj/ff matmuls on TensorE with PE-transposed activations as stationary;
    the attn residual (x, bf16) is accumulated into the proj PSUM via an
    identity matmul so z1 needs no VectorE PSUM-read add.
  - LayerNorm stats on ScalarE via activation accum_out (Identity/Square);
    rsigma = exp(-0.5*ln(var+eps)) so softmax-exp and LN share one ACT table
    set; ln_g/ln_b of LN1 are folded into the ff weights on the host; LN2's
    affine is skipped entirely when ln_g==1 and ln_b==0 (program variant).
  - engine placement discipline: keeping ScalarE/GpSimd co-activity low
    matters more than offloading VectorE — heavy co-activity inflates every
    engine's per-op time ~20% (SBUF contention / power throttle).
"""

import sys

sys.path.insert(0, "/opt/trn_rl_repo")

import numpy as np
import ml_dtypes

E, H, DQ, DV = 1024, 16, 64, 64
B, S = 8, 4096
EPS = 1e-5
NT = S // 128  # 32 position tiles per core
BF = ml_dtypes.bfloat16
F8 = ml_dtypes.float8_e4m3
W8SCALE = 32.0  # qkv weights are ~1/32; prescale into fp8's normal range

_CACHE = {}


def _patch_tail_drain():
    """walrus in this container rejects >1 sem wait on a CTRL (Drain)
    instruction; spread the TileContext tail-drain waits over wait-nops."""
    import concourse.tile as tile
    import bass_rust
    from concourse.vector_clock import ScopedClock

    if getattr(tile.TileContext, "_drain_patched", False):
        return

    def _drain_and_barrier(self, tick_clock, wait_clock):
        nc = self.nc
        drain_inst = nc.sync.drain()
        wait_clock.add_sem_waits(
            drain_inst.ins, ScopedClock({None: tick_clock.global_clock})
        )
        si = drain_inst.ins.sync_info
        waits = list(si.on_wait) if si is not None else []
        if len(waits) > 1:
            drain_inst.ins.sync_info = bass_rust.SyncInfo(on_wait=[], on_update=[])
            for w in waits:
                nop = nc.sync.nop()
                nop.ins.sync_info = bass_rust.SyncInfo(on_wait=[w], on_update=[])
        nc.all_engine_barrier()
        assert self.sems is not None
        popped = nc._tile_sem_poison_stack.pop()
        assert popped is self._sem_poison
        nc.clear_and_free_semaphores(list(self.sems.allocated().values()))
        nc.all_engine_barrier()

    tile.TileContext._drain_and_barrier = _drain_and_barrier
    tile.TileContext._drain_patched = True


def _split_excess_waits(nc, max_on_op=1, max_on_nop=1):
    """walrus in this container rejects >2 sem waits on compute instruction
    structs and >1 on DMA/CTRL structs. Hoist excess waits onto preceding
    same-engine NOPs."""
    import concourse.mybir as mybir
    import bass_rust

    narrow = {"DMACopy", "Drain", "NoOp", "Memset", "TriggeredCopy"}
    cnt = 0
    for bb in nc.m.functions[0].blocks:
        il = bb.instructions
        out = []
        for inst in il:
            cap = 1 if inst.opcode in narrow else max_on_op
            si = inst.sync_info
            waits = list(si.on_wait) if si is not None and si.on_wait else []
            if len(waits) > cap:
                n_extra = len(waits) - cap
                extra, keep = waits[:n_extra], waits[n_extra:]
                for i0 in range(0, len(extra), max_on_nop):
                    chunk = extra[i0 : i0 + max_on_nop]
                    nop = mybir.InstNoOp(name=f"waitnop-{cnt}", ins=[], outs=[])
                    cnt += 1
                    nop.engine = inst.engine
                    nop.sync_info = bass_rust.SyncInfo(on_wait=chunk, on_update=[])
                    out.append(nop)
                inst.sync_info = bass_rust.SyncInfo(
                    on_wait=keep,
                    on_update=list(si.on_update) if si.on_update else [],
                )
            out.append(inst)
        il[:] = out


def _build_program(trivial_affine: bool, variant: int = 0):
    import concourse.bass as bass
    import concourse.tile as tile
    import concourse.mybir as mybir
    from concourse.masks import make_identity

    _patch_tail_drain()

    f32 = mybir.dt.float32
    bf16 = mybir.dt.bfloat16
    fp8 = mybir.dt.float8e4
    ALU = mybir.AluOpType
    ACT = mybir.ActivationFunctionType
    DR = mybir.MatmulPerfMode.DoubleRow

    nc = bass.Bass("TRN2", target_bir_lowering=False, debug=False, num_devices=1)

    if variant < 7:
        x_pm = nc.dram_tensor("x_pm", [S, E], f32, kind="ExternalInput").ap()
    x_bf_d = nc.dram_tensor("x_bf", [S, E], bf16, kind="ExternalInput").ap()
    if variant >= 2:
        xsum_d = nc.dram_tensor("xsum", [S, 1], f32, kind="ExternalInput").ap()
    xT = nc.dram_tensor("xT", [E, S], fp8, kind="ExternalInput").ap()
    wqkvT_d = nc.dram_tensor("wqkvT", [E, 3 * E], fp8, kind="ExternalInput").ap()
    projT_d = nc.dram_tensor("projT", [E, E], bf16, kind="ExternalInput").ap()
    ffw2T_d = nc.dram_tensor("ffw2T", [E, E], bf16, kind="ExternalInput").ap()
    bqkv_d = nc.dram_tensor("bqkv", [1, 3 * E], bf16, kind="ExternalInput").ap()
    bproj_d = nc.dram_tensor("bproj", [1, E], bf16, kind="ExternalInput").ap()
    bff2_d = nc.dram_tensor("bff2", [1, E], bf16, kind="ExternalInput").ap()
    if not trivial_affine:
        g_rep_d = nc.dram_tensor("g_rep", [128, E], f32, kind="ExternalInput").ap()
        b_rep_d = nc.dram_tensor("b_rep", [128, E], f32, kind="ExternalInput").ap()
    out_d = nc.dram_tensor("out", [S, E], f32, kind="ExternalOutput").ap()

    xT_r = xT.rearrange("(t p) s -> p t s", p=128)  # [128, 8, S]
    wqkv_r = wqkvT_d.rearrange("(t p) o -> p t o", p=128)
    proj_r = projT_d.rearrange("(t p) o -> p t o", p=128)
    ffw2_r = ffw2T_d.rearrange("(t p) o -> p t o", p=128)

    with tile.TileContext(nc) as tc:
        import contextlib

        ctx = contextlib.ExitStack()
        with ctx:
            fixed = ctx.enter_context(tc.tile_pool(name="fixed", bufs=1))
            work = ctx.enter_context(
                tc.tile_pool(name="work", bufs=(4 if variant >= 6 else 3))
            )
            work1 = ctx.enter_context(tc.tile_pool(name="work1", bufs=1))
            if variant >= 10:
                # qkv double-buffered: the qkv stage runs one tile ahead so
                # the next tile's first prod never waits on ScalarE evictions
                qkvp = ctx.enter_context(tc.tile_pool(name="qkvp", bufs=2))
            stats = ctx.enter_context(tc.tile_pool(name="stats", bufs=8))
            psq = ctx.enter_context(
                tc.tile_pool(name="psq", bufs=(2 if variant >= 9 else 3), space="PSUM")
            )
            pst = ctx.enter_context(tc.tile_pool(name="pst", bufs=2, space="PSUM"))
            if variant >= 9:
                # 2-bank [128,1024] f32 tiles: proj and ff each evict in ONE
                # ScalarE pass (one accum read) instead of two + a pair-add
                psb = ctx.enter_context(tc.tile_pool(name="psz", bufs=2, space="PSUM"))
            else:
                psb = ctx.enter_context(
                    tc.tile_pool(
                        name="psb", bufs=(3 if variant >= 4 else 2), space="PSUM"
                    )
                )

            # ---- fixed tensors ----
            # v7: tile 0's x loads are hoisted ahead of the weight loads so
            # compute opens ~1.5MB into the DMA stream instead of ~7MB.
            wdma = nc.sync.dma_start
            pre = {}
            n_pre = 2 if variant >= 9 else 1
            if variant >= 7:
                # first tiles' inputs first: compute can start after ~0.7MB
                for pt_ in range(n_pre):
                    o = pt_ * 128
                    pre_xbf = work.tile([128, E], bf16, tag="xbf")
                    nc.sync.dma_start(out=pre_xbf, in_=x_bf_d[o : o + 128, :])
                    pre_xs = work.tile([128, 1], f32, tag="xs")
                    nc.sync.dma_start(out=pre_xs, in_=xsum_d[o : o + 128, :])
                    pre_xf = work.tile([128, 8, 128], fp8, tag="xf")
                    nc.sync.dma_start(out=pre_xf, in_=xT_r[:, :, o : o + 128])
                    pre[pt_] = (pre_xbf, pre_xs, pre_xf)
            bqkv_sb = fixed.tile([1, 3 * E], bf16)
            wdma(out=bqkv_sb, in_=bqkv_d)
            wqkv_sb = fixed.tile([128, 8, 3 * E], fp8)
            if variant:
                # column-group order: tile 0's first psum chunk only waits on
                # the first 1/6th of the weight load
                for j in range(6):
                    wdma(
                        out=wqkv_sb[:, :, j * 512 : (j + 1) * 512],
                        in_=wqkv_r[:, :, j * 512 : (j + 1) * 512],
                    )
            else:
                for t in range(8):
                    wdma(out=wqkv_sb[:, t, :], in_=wqkv_r[:, t, :])
            # v10: proj/ff weights ride the (otherwise idle) GpSimd SWDGE
            # queue so ~18 trigger slots don't serialize the sync queue at
            # startup in front of the per-tile x loads
            w2dma = nc.gpsimd.dma_start if variant == 10 else wdma
            proj_sb = fixed.tile([128, 8, E], bf16)
            ffw2_sb = fixed.tile([128, 8, E], bf16)
            for t in range(8):
                w2dma(out=proj_sb[:, t, :], in_=proj_r[:, t, :])
            for t in range(8):
                w2dma(out=ffw2_sb[:, t, :], in_=ffw2_r[:, t, :])
            bproj_sb = fixed.tile([1, E], bf16)
            w2dma(out=bproj_sb, in_=bproj_d)
            bff2_sb = fixed.tile([1, E], bf16)
            w2dma(out=bff2_sb, in_=bff2_d)
            if not trivial_affine:
                g_rep = fixed.tile([128, E], f32)
                nc.sync.dma_start(out=g_rep, in_=g_rep_d)
                b_rep = fixed.tile([128, E], f32)
                nc.sync.dma_start(out=b_rep, in_=b_rep_d)
            ones_row = fixed.tile([1, 128], bf16)
            nc.vector.memset(ones_row, 1.0)
            ident = fixed.tile([128, 128], bf16)
            make_identity(nc, ident)
            eps_sb = fixed.tile([128, 1], f32)
            nc.vector.memset(eps_sb, EPS)

            inv_n = 1.0 / float(E)

            def emit_tail(gl_p, xp_p, s0_p, s1z2=None):
                """Deferred tile tail: z2 = gelu_out + x, LN2, affine, store.
                Emitted one iteration late so the DVE z2/LN ops land in the
                next tile's exp-wait window instead of stalling on gelu."""
                if variant >= 9:
                    # gl_p is the ff PSUM [128,1024] (2 banks); the gelu
                    # eviction itself is deferred to here so it lands right
                    # after the next tile's softmax exp in ScalarE's FIFO
                    # (the gelu ACT-table swap then sits off-critical too).
                    # s1z2 arrives as the xs tile: sum(z2) = sum(gelu) + sum(x).
                    gl = work.tile([128, E], bf16, tag="gl")
                    sgl = stats.tile([128, 1], f32, tag="sgl")
                    nc.scalar.activation(gl, gl_p, ACT.Gelu, accum_out=sgl)
                    s1t = stats.tile([128, 1], f32, tag="s1z2")
                    nc.scalar.activation(s1t, sgl, ACT.Identity, bias=s1z2)
                    s1z2 = s1t
                    gl_p = gl
                if variant >= 7:
                    # bf16 residual add: 2x DVE mode, and the f32 x load is
                    # dropped entirely (xp_p is the bf16 x tile here)
                    z2 = work.tile([128, E], bf16, tag="z2")
                    nc.vector.tensor_tensor(z2, gl_p, xp_p, ALU.add)
                else:
                    z2 = work.tile([128, E], f32, tag="z2")
                    for j in range(2):
                        nc.vector.tensor_tensor(
                            z2[:, j * 512 : (j + 1) * 512],
                            gl_p[:, j * 512 : (j + 1) * 512],
                            xp_p[:, j * 512 : (j + 1) * 512],
                            ALU.add,
                        )
                rs2 = stats.tile([128, 1], f32, tag="rs2")
                mrs2 = stats.tile([128, 1], f32, tag="mrs2")
                lnscr2 = work1.tile([128, E], bf16, tag="lnscr2")
                layer_norm(z2, rs2, mrs2, lnscr2, s1_pre=s1z2)
                if trivial_affine:
                    out_t = work.tile([128, E], f32, tag="out_t")
                    nc.scalar.activation(
                        out_t, z2, ACT.Identity, bias=mrs2, scale=rs2
                    )
                else:
                    zn = work1.tile([128, E], f32, tag="zn")
                    nc.scalar.activation(zn, z2, ACT.Identity, bias=mrs2, scale=rs2)
                    zn2 = work1.tile([128, E], f32, tag="zn2")
                    nc.gpsimd.tensor_tensor(zn2, zn, g_rep, ALU.mult)
                    out_t = work.tile([128, E], f32, tag="out_t")
                    nc.gpsimd.tensor_tensor(out_t, zn2, b_rep, ALU.add)
                nc.sync.dma_start(out=out_d[s0_p : s0_p + 128, :], in_=out_t)

            pending = None

            def layer_norm_scalar(s1, s2, rs_out, mrs_out):
                """LN scalar chain entirely on ScalarE: keeps every [P,1] op
                out of DVE's in-order queue so DVE never head-of-line blocks
                on cross-engine stats. rsigma = exp(-0.5*ln(s2/N + eps - mu^2))."""
                mneg = stats.tile([128, 1], f32, tag="mneg")
                nc.scalar.activation(mneg, s1, ACT.Identity, scale=-inv_n)
                mu2 = stats.tile([128, 1], f32, tag="mu2")
                nc.scalar.activation(mu2, s1, ACT.Square, scale=inv_n)
                emm = stats.tile([128, 1], f32, tag="emm")
                nc.scalar.activation(emm, mu2, ACT.Identity, scale=-1.0, bias=eps_sb)
                lnv = stats.tile([128, 1], f32, tag="lnv")
                nc.scalar.activation(lnv, s2, ACT.Ln, scale=inv_n, bias=emm)
                nc.scalar.activation(rs_out, lnv, ACT.Exp, scale=-0.5)
                nc.scalar.activation(mrs_out, mneg, ACT.Identity, scale=rs_out)

            def layer_norm(z, rs_out, mrs_out, scratch_bf, s1_pre=None, s2_pre=None):
                """Compute rsigma and -mu*rsigma of z [128, E] (fp32).
                If s1_pre/s2_pre are given (sum / sum-of-squares precomputed,
                e.g. fused into the PSUM eviction), those passes are skipped."""
                if s1_pre is None:
                    s1 = stats.tile([128, 1], f32, tag="s1")
                    nc.scalar.activation(scratch_bf, z, ACT.Identity, accum_out=s1)
                else:
                    s1 = s1_pre
                if s2_pre is None:
                    s2 = stats.tile([128, 1], f32, tag="s2")
                    nc.scalar.activation(scratch_bf, z, ACT.Square, accum_out=s2)
                else:
                    s2 = s2_pre
                if variant >= 8:
                    layer_norm_scalar(s1, s2, rs_out, mrs_out)
                    return
                mu = stats.tile([128, 1], f32, tag="mu")
                lnv = stats.tile([128, 1], f32, tag="lnv")
                nc.vector.tensor_scalar_mul(mu, s1, inv_n)
                mu2 = stats.tile([128, 1], f32, tag="mu2")
                nc.vector.tensor_tensor(mu2, mu, mu, ALU.mult)
                var = stats.tile([128, 1], f32, tag="var")
                nc.vector.scalar_tensor_tensor(
                    var, in0=s2, scalar=inv_n, in1=mu2, op0=ALU.mult, op1=ALU.subtract
                )
                nc.scalar.activation(lnv, var, ACT.Ln, bias=eps_sb)
                nc.scalar.activation(rs_out, lnv, ACT.Exp, scale=-0.5)
                nc.vector.scalar_tensor_tensor(
                    mrs_out, in0=mu, scalar=-1.0, in1=rs_out, op0=ALU.mult, op1=ALU.mult
                )

            def qkv_stage(t):
                """x loads + qkv projection for tile t; returns (xbf, xs, qkv_sb).
                For variant>=10 this runs one tile AHEAD of the main body."""
                s0 = t * 128
                if t in pre:
                    xbf, xs, xf = pre[t]
                else:
                    xbf = work.tile([128, E], bf16, tag="xbf")
                    nc.sync.dma_start(out=xbf, in_=x_bf_d[s0 : s0 + 128, :])
                    xs = work.tile([128, 1], f32, tag="xs")
                    nc.sync.dma_start(out=xs, in_=xsum_d[s0 : s0 + 128, :])
                    xf = work.tile([128, 8, 128], fp8, tag="xf")
                    nc.sync.dma_start(out=xf, in_=xT_r[:, :, s0 : s0 + 128])
                qkv_sb = qkvp.tile([128, 3 * E], bf16, tag="qkv")
                for j in range(6):
                    ps = psq.tile([128, 512], f32, tag="psq")
                    for e2 in range(4):
                        nc.tensor.matmul(
                            ps,
                            xf[:, 2 * e2 : 2 * e2 + 2, :],
                            wqkv_sb[:, 2 * e2 : 2 * e2 + 2, j * 512 : (j + 1) * 512],
                            start=(e2 == 0),
                            stop=False,
                            perf_mode=DR,
                        )
                    nc.tensor.matmul(
                        ps,
                        ones_row,
                        bqkv_sb[:, j * 512 : (j + 1) * 512],
                        start=False,
                        stop=True,
                    )
                    nc.scalar.mul(qkv_sb[:, j * 512 : (j + 1) * 512], ps, 1.0 / W8SCALE)
                return xbf, xs, qkv_sb

            cur = None
            for t in range(NT):
                s0 = t * 128
                if variant >= 10:
                    xp = None
                    if t == 0:
                        cur = qkv_stage(0)
                    xbf, xs, qkv_sb = cur
                elif variant >= 7:
                    xp = None
                    if t in pre:
                        xbf, xs, xf = pre[t]
                    else:
                        xbf = work.tile([128, E], bf16, tag="xbf")
                        nc.sync.dma_start(out=xbf, in_=x_bf_d[s0 : s0 + 128, :])
                        xs = work.tile([128, 1], f32, tag="xs")
                        nc.sync.dma_start(out=xs, in_=xsum_d[s0 : s0 + 128, :])
                        xf = work.tile([128, 8, 128], fp8, tag="xf")
                        nc.sync.dma_start(out=xf, in_=xT_r[:, :, s0 : s0 + 128])
                else:
                    xp = work.tile([128, E], f32, tag="xp")
                    nc.sync.dma_start(out=xp, in_=x_pm[s0 : s0 + 128, :])
                    xbf = work.tile([128, E], bf16, tag="xbf")
                    nc.sync.dma_start(out=xbf, in_=x_bf_d[s0 : s0 + 128, :])
                    if variant >= 2:
                        xs = work.tile([128, 1], f32, tag="xs")
                        nc.sync.dma_start(out=xs, in_=xsum_d[s0 : s0 + 128, :])
                    xf = work.tile([128, 8, 128], fp8, tag="xf")
                    nc.sync.dma_start(out=xf, in_=xT_r[:, :, s0 : s0 + 128])

                if variant < 10:
                    # ---- qkv projection (fp8 DoubleRow, weights prescaled x32) ----
                    qkv_sb = work1.tile([128, 3 * E], bf16, tag="qkv")
                    for j in range(6):
                        ps = psq.tile([128, 512], f32, tag="psq")
                        for e2 in range(4):
                            nc.tensor.matmul(
                                ps,
                                xf[:, 2 * e2 : 2 * e2 + 2, :],
                                wqkv_sb[:, 2 * e2 : 2 * e2 + 2, j * 512 : (j + 1) * 512],
                                start=(e2 == 0),
                                stop=False,
                                perf_mode=DR,
                            )
                        nc.tensor.matmul(
                            ps,
                            ones_row,
                            bqkv_sb[:, j * 512 : (j + 1) * 512],
                            start=False,
                            stop=True,
                        )
                        nc.scalar.mul(
                            qkv_sb[:, j * 512 : (j + 1) * 512], ps, 1.0 / W8SCALE
                        )

                q3 = qkv_sb[:, 0:E].rearrange("p (h d) -> p h d", h=H)
                k3 = qkv_sb[:, E : 2 * E].rearrange("p (g d) -> p g d", g=H)
                v3 = qkv_sb[:, 2 * E : 3 * E].rearrange("p (d g) -> p d g", d=DV)

                # ---- QK^T scores ----
                prod = work1.tile([128, 8, 16, 64], bf16, tag="prod")
                scr = work1.tile([128, 8192], bf16, tag="scr")
                scores = work.tile([128, H, H], bf16 if variant >= 8 else f32, tag="scores")
                p_sb = work.tile([128, H, H], bf16, tag="p_sb")
                for half in range(2):
                    h0 = half * 8
                    qb = q3[:, h0 : h0 + 8, :].unsqueeze(2).broadcast_to([128, 8, 16, 64])
                    kb = k3.unsqueeze(1).broadcast_to([128, 8, 16, 64])
                    nc.vector.tensor_tensor(prod, kb, qb, ALU.mult)
                    t1 = scr[:, 0:4096].rearrange("p (a g d) -> p a g d", a=8, g=16)
                    nc.vector.tensor_tensor(
                        t1, prod[:, :, :, 0:32], prod[:, :, :, 32:64], ALU.add
                    )
                    t2 = scr[:, 4096:6144].rearrange("p (a g d) -> p a g d", a=8, g=16)
                    nc.vector.tensor_tensor(
                        t2, t1[:, :, :, 0:16], t1[:, :, :, 16:32], ALU.add
                    )
                    t3 = scr[:, 6144:7168].rearrange("p (a g d) -> p a g d", a=8, g=16)
                    nc.vector.tensor_tensor(
                        t3, t2[:, :, :, 0:8], t2[:, :, :, 8:16], ALU.add
                    )
                    t4 = scr[:, 7168:7680].rearrange("p (a g d) -> p a g d", a=8, g=16)
                    nc.vector.tensor_tensor(
                        t4, t3[:, :, :, 0:4], t3[:, :, :, 4:8], ALU.add
                    )
                    if variant >= 8:
                        # finish with 2x-mode TT halvings instead of a 1x reduce
                        t5 = scr[:, 7680:7936].rearrange(
                            "p (a g d) -> p a g d", a=8, g=16
                        )
                        nc.vector.tensor_tensor(
                            t5, t4[:, :, :, 0:2], t4[:, :, :, 2:4], ALU.add
                        )
                        nc.vector.tensor_tensor(
                            scores[:, h0 : h0 + 8, :].unsqueeze(3),
                            t5[:, :, :, 0:1],
                            t5[:, :, :, 1:2],
                            ALU.add,
                        )
                    else:
                        nc.vector.tensor_reduce(
                            scores[:, h0 : h0 + 8, :],
                            t4,
                            axis=mybir.AxisListType.X,
                            op=ALU.add,
                        )

                # ---- softmax (no max-subtraction; fold 1/den into p before AV) ----
                nc.scalar.activation(p_sb, scores, ACT.Exp)
                if pending is not None and variant < 9:
                    emit_tail(*pending)
                    pending = None
                den = stats.tile([128, H], f32, tag="den")
                nc.vector.tensor_reduce(
                    den, p_sb, axis=mybir.AxisListType.X, op=ALU.add
                )
                rden = stats.tile([128, H], f32, tag="rden")
                nc.vector.reciprocal(rden, den)
                p_nm = work.tile([128, H, H], bf16, tag="p_nm")
                nc.vector.tensor_tensor(
                    p_nm,
                    p_sb,
                    rden.unsqueeze(2).broadcast_to([128, H, H]),
                    ALU.mult,
                )
                if pending is not None:
                    # v9: tail emitted after den/rden/p_nm so DVE's in-order
                    # queue hits den (waits only on exp) before z2 (waits on
                    # the deferred gelu eviction)
                    emit_tail(*pending)
                    pending = None
                if variant >= 10 and t + 1 < NT:
                    # next tile's qkv stage here: its ScalarE evictions land
                    # ahead of this tile's LN/eviction stream, so the next
                    # tile's first prod has its inputs a full tile early
                    cur = qkv_stage(t + 1)

                # ---- attn @ v ----
                attn_bf = work.tile([128, E], bf16, tag="attn_bf")
                a3 = attn_bf.rearrange("p (h d) -> p h d", h=H)
                prod_flat = prod.rearrange("p a g d -> p (a g d)")
                for half in range(2):
                    h0 = half * 8
                    # reuse prod's memory with a contiguous [128, 8, 64, 16] layout
                    pa = prod_flat.rearrange("p (a d g) -> p a d g", a=8, d=DV)
                    pb = (
                        p_nm[:, h0 : h0 + 8, :]
                        .unsqueeze(2)
                        .broadcast_to([128, 8, 64, 16])
                    )
                    vb = v3.unsqueeze(1).broadcast_to([128, 8, 64, 16])
                    nc.vector.tensor_tensor(pa, vb, pb, ALU.mult)
                    u1 = scr[:, 0:4096].rearrange("p (a d g) -> p a d g", a=8, d=64)
                    nc.vector.tensor_tensor(
                        u1, pa[:, :, :, 0:8], pa[:, :, :, 8:16], ALU.add
                    )
                    u2 = scr[:, 4096:6144].rearrange("p (a d g) -> p a d g", a=8, d=64)
                    nc.vector.tensor_tensor(
                        u2, u1[:, :, :, 0:4], u1[:, :, :, 4:8], ALU.add
                    )
                    u3 = scr[:, 6144:7168].rearrange("p (a d g) -> p a d g", a=8, d=64)
                    nc.vector.tensor_tensor(
                        u3, u2[:, :, :, 0:2], u2[:, :, :, 2:4], ALU.add
                    )
                    nc.vector.tensor_tensor(
                        a3[:, h0 : h0 + 8, :].unsqueeze(3),
                        u3[:, :, :, 0:1],
                        u3[:, :, :, 1:2],
                        ALU.add,
                    )

                # ---- transpose attn_out to feature-major (4 or 8 per PSUM tile) ----
                # v7: PSUM->SBUF evictions on ScalarE (DVE is the bottleneck)
                evict = nc.scalar.copy if variant >= 7 else nc.vector.tensor_copy
                attn_fm = work.tile([128, 8, 128], bf16, tag="attn_fm")
                TG = 8 if variant >= 5 else 4
                for q in range(8 // TG):
                    pt = pst.tile([128, TG * 128], bf16, tag="pst")
                    for e4 in range(TG):
                        e = q * TG + e4
                        nc.tensor.transpose(
                            pt[:, e4 * 128 : (e4 + 1) * 128],
                            attn_bf[:, e * 128 : (e + 1) * 128],
                            ident,
                        )
                    evict(
                        attn_fm[:, q * TG : (q + 1) * TG, :].rearrange(
                            "p a b -> p (a b)"
                        ),
                        pt,
                    )

                # ---- proj + residual (x folded in via identity matmul) ----
                z1 = work1.tile([128, E], f32, tag="z1")
                lnscr = work1.tile([128, E], bf16, tag="lnscr")
                s1parts = []
                s2parts = []
                ps2w = None
                for j in range(2):
                    if variant >= 9:
                        if j == 0:
                            ps2w = psb.tile([128, 1024], f32, tag="psz")
                        ps2 = ps2w[:, j * 512 : (j + 1) * 512]
                    else:
                        ps2 = psb.tile([128, 512], f32, tag="psb")
                    for e in range(8):
                        nc.tensor.matmul(
                            ps2,
                            attn_fm[:, e, :],
                            proj_sb[:, e, j * 512 : (j + 1) * 512],
                            start=(e == 0),
                            stop=False,
                        )
                    nc.tensor.matmul(
                        ps2,
                        ones_row,
                        bproj_sb[:, j * 512 : (j + 1) * 512],
                        start=False,
                        stop=False,
                    )
                    nc.tensor.matmul(
                        ps2,
                        ident,
                        xbf[:, j * 512 : (j + 1) * 512],
                        start=False,
                        stop=True,
                    )
                    if variant >= 9:
                        continue
                    if variant:
                        # fuse the sum-of-z1 accumulation into the eviction
                        s1p = stats.tile([128, 1], f32, tag=f"s1p{j}")
                        s1parts.append(s1p)
                        nc.scalar.activation(
                            z1[:, j * 512 : (j + 1) * 512],
                            ps2,
                            ACT.Identity,
                            accum_out=s1p,
                        )
                        if variant >= 3:
                            # sum-of-squares per chunk straight from PSUM too
                            s2p = stats.tile([128, 1], f32, tag=f"s2p{j}")
                            s2parts.append(s2p)
                            nc.scalar.activation(
                                lnscr[:, j * 512 : (j + 1) * 512],
                                ps2,
                                ACT.Square,
                                accum_out=s2p,
                            )
                    else:
                        nc.scalar.copy(z1[:, j * 512 : (j + 1) * 512], ps2)

                # ---- LN1 (g,b folded into ff weights) ----
                rs1 = stats.tile([128, 1], f32, tag="rs1")
                mrs1 = stats.tile([128, 1], f32, tag="mrs1")
                if variant >= 9:
                    # single [128,1024] eviction across both PSUM banks: one
                    # accum read per stat, no pair-adds
                    s1p = stats.tile([128, 1], f32, tag="s1p")
                    nc.scalar.activation(z1, ps2w, ACT.Identity, accum_out=s1p)
                    s2p = stats.tile([128, 1], f32, tag="s2p")
                    nc.scalar.activation(lnscr, ps2w, ACT.Square, accum_out=s2p)
                    layer_norm(z1, rs1, mrs1, lnscr, s1_pre=s1p, s2_pre=s2p)
                elif variant:
                    s1f = stats.tile([128, 1], f32, tag="s1f")
                    if variant >= 7:
                        nc.scalar.activation(
                            s1f, s1parts[0], ACT.Identity, bias=s1parts[1]
                        )
                    else:
                        nc.vector.tensor_tensor(s1f, s1parts[0], s1parts[1], ALU.add)
                    if variant >= 3:
                        s2f = stats.tile([128, 1], f32, tag="s2f")
                        if variant >= 7:
                            nc.scalar.activation(
                                s2f, s2parts[0], ACT.Identity, bias=s2parts[1]
                            )
                        else:
                            nc.vector.tensor_tensor(
                                s2f, s2parts[0], s2parts[1], ALU.add
                            )
                        layer_norm(z1, rs1, mrs1, lnscr, s1_pre=s1f, s2_pre=s2f)
                    else:
                        layer_norm(z1, rs1, mrs1, lnscr, s1_pre=s1f)
                else:
                    layer_norm(z1, rs1, mrs1, lnscr)
                ln1_bf = work.tile([128, E], bf16, tag="ln1_bf")
                nc.scalar.activation(ln1_bf, z1, ACT.Identity, bias=mrs1, scale=rs1)

                ln1_fm = work.tile([128, 8, 128], bf16, tag="ln1_fm")
                for q in range(8 // TG):
                    pt = pst.tile([128, TG * 128], bf16, tag="pst")
                    for e4 in range(TG):
                        e = q * TG + e4
                        nc.tensor.transpose(
                            pt[:, e4 * 128 : (e4 + 1) * 128],
                            ln1_bf[:, e * 128 : (e + 1) * 128],
                            ident,
                        )
                    evict(
                        ln1_fm[:, q * TG : (q + 1) * TG, :].rearrange(
                            "p a b -> p (a b)"
                        ),
                        pt,
                    )

                # ---- ff + gelu (z2/LN2 deferred to next iteration's tail) ----
                if variant >= 9:
                    # matmuls only; the gelu eviction is deferred into the
                    # next iteration's tail (after its softmax exp)
                    ps3w = psb.tile([128, 1024], f32, tag="psz")
                    for j in range(2):
                        pj = ps3w[:, j * 512 : (j + 1) * 512]
                        for e in range(8):
                            nc.tensor.matmul(
                                pj,
                                ln1_fm[:, e, :],
                                ffw2_sb[:, e, j * 512 : (j + 1) * 512],
                                start=(e == 0),
                                stop=False,
                            )
                        nc.tensor.matmul(
                            pj,
                            ones_row,
                            bff2_sb[:, j * 512 : (j + 1) * 512],
                            start=False,
                            stop=True,
                        )
                    pending = (ps3w, xbf, s0, xs)
                    continue
                gl = work.tile([128, E], bf16 if variant >= 7 else f32, tag="gl")
                sglparts = []
                for j in range(2):
                    ps3 = psb.tile([128, 512], f32, tag="psb")
                    for e in range(8):
                        nc.tensor.matmul(
                            ps3,
                            ln1_fm[:, e, :],
                            ffw2_sb[:, e, j * 512 : (j + 1) * 512],
                            start=(e == 0),
                            stop=False,
                        )
                    nc.tensor.matmul(
                        ps3,
                        ones_row,
                        bff2_sb[:, j * 512 : (j + 1) * 512],
                        start=False,
                        stop=True,
                    )
                    if variant >= 2:
                        # fuse the sum-of-gelu accumulation into the eviction;
                        # sum(z2) = sum(gelu) + sum(x) (host-precomputed xsum)
                        sgl = stats.tile([128, 1], f32, tag=f"sgl{j}")
                        sglparts.append(sgl)
                        nc.scalar.activation(
                            gl[:, j * 512 : (j + 1) * 512],
                            ps3,
                            ACT.Gelu,
                            accum_out=sgl,
                        )
                    else:
                        nc.scalar.activation(
                            gl[:, j * 512 : (j + 1) * 512], ps3, ACT.Gelu
                        )

                if variant == 8:
                    # prefetch the exp/ln ACT table now (gelu swapped it out),
                    # so the next tile's softmax exp isn't stuck behind a
                    # 1.3us ACT_TABLE_LOAD on the critical path
                    atld = stats.tile([128, 1], f32, tag="atld")
                    nc.scalar.activation(atld, eps_sb, ACT.Ln)
                    atld2 = stats.tile([128, 1], f32, tag="atld2")
                    nc.scalar.activation(atld2, eps_sb, ACT.Exp)

                if variant >= 7:
                    # [P,1] pair-adds ride ScalarE (bias is added pre-func)
                    sgf = stats.tile([128, 1], f32, tag="sgf")
                    nc.scalar.activation(
                        sgf, sglparts[0], ACT.Identity, bias=sglparts[1]
                    )
                    s1z2 = stats.tile([128, 1], f32, tag="s1z2")
                    nc.scalar.activation(s1z2, sgf, ACT.Identity, bias=xs)
                    pending = (gl, xbf, s0, s1z2)
                elif variant >= 2:
                    sgf = stats.tile([128, 1], f32, tag="sgf")
                    nc.vector.tensor_tensor(sgf, sglparts[0], sglparts[1], ALU.add)
                    s1z2 = stats.tile([128, 1], f32, tag="s1z2")
                    nc.vector.tensor_tensor(s1z2, sgf, xs, ALU.add)
                    pending = (gl, xp, s0, s1z2)
                else:
                    pending = (gl, xp, s0)

            emit_tail(*pending)

    _split_excess_waits(nc)
    return nc


def _host_prep(inputs, trivial_affine=None):
    x = np.asarray(inputs["x"], np.float32)
    qk_w = np.asarray(inputs["qk_w"], np.float32)
    qk_b = np.asarray(inputs["qk_b"], np.float32)
    v_w = np.asarray(inputs["v_w"], np.float32)
    v_b = np.asarray(inputs["v_b"], np.float32)
    proj_w = np.asarray(inputs["proj_w"], np.float32)
    proj_b = np.asarray(inputs["proj_b"], np.float32)
    ff_w = np.asarray(inputs["ff_w"], np.float32)
    ff_b = np.asarray(inputs["ff_b"], np.float32)
    ln_g = np.asarray(inputs["ln_g"], np.float32)
    ln_b = np.asarray(inputs["ln_b"], np.float32)

    if trivial_affine is None:
        trivial_affine = bool(
            np.allclose(ln_g, 1.0, atol=1e-7) and np.allclose(ln_b, 0.0, atol=1e-7)
        )

    scale = 1.0 / np.sqrt(DQ).astype(np.float32)
    Wq = qk_w[:E] * scale
    bq = qk_b[:E] * scale
    Wk = qk_w[E:]
    bk = qk_b[E:]
    g_idx, d_idx = np.meshgrid(np.arange(H), np.arange(DV), indexing="ij")
    perm = np.empty(E, np.int64)
    perm[(d_idx * H + g_idx).ravel()] = (g_idx * DV + d_idx).ravel()
    Wv2 = v_w[perm]
    bv2 = v_b[perm]

    wqkvT = np.ascontiguousarray(
        (np.concatenate([Wq, Wk, Wv2], 0) * W8SCALE).T.astype(F8)
    )  # [E, 3E] fp8, prescaled
    bqkv = (np.concatenate([bq, bk, bv2]) * W8SCALE)[None, :].astype(BF)  # [1, 3E]
    projT = np.ascontiguousarray(proj_w.T.astype(BF))  # [E, E]
    bproj = proj_b[None, :].astype(BF)
    ffw2T = np.ascontiguousarray((ff_w * ln_g[None, :]).T.astype(BF))
    bff2 = (ff_b + ff_w @ ln_b)[None, :].astype(BF)

    shared = {
        "wqkvT": wqkvT,
        "bqkv": bqkv,
        "projT": projT,
        "bproj": bproj,
        "ffw2T": ffw2T,
        "bff2": bff2,
    }
    if not trivial_affine:
        shared["g_rep"] = np.ascontiguousarray(
            np.broadcast_to(ln_g[None, :], (128, E)), np.float32
        )
        shared["b_rep"] = np.ascontiguousarray(
            np.broadcast_to(ln_b[None, :], (128, E)), np.float32
        )
    in_maps = []
    for b in range(B):
        xb = np.ascontiguousarray(x[b])  # [S, E] f32
        xTb = np.ascontiguousarray(xb.T.astype(F8))  # [E, S] fp8
        m = {
            "x_pm": xb,
            "x_bf": xb.astype(BF),
            "xT": xTb,
            "xsum": np.ascontiguousarray(xb.sum(-1, dtype=np.float32)[:, None]),
        }
        m.update(shared)
        in_maps.append(m)
    return in_maps


def kernel(**inputs) -> np.ndarray:
    from concourse.bass_utils import run_bass_kernel_spmd

    trivial_affine = bool(
        np.allclose(np.asarray(inputs["ln_g"]), 1.0, atol=1e-7)
        and np.allclose(np.asarray(inputs["ln_b"]), 0.0, atol=1e-7)
    )
    variant = 11  # v10 but weight loads back on the sync HWDGE queue
    key = ("nc", trivial_affine, variant)
    if key not in _CACHE:
        _CACHE[key] = _build_program(trivial_affine, variant)
    nc = _CACHE[key]

    in_maps = _host_prep(inputs, trivial_affine)
    res = run_bass_kernel_spmd(nc, in_maps, core_ids=list(range(B)))
    out = np.stack([res.results[b]["out"] for b in range(B)], 0)
    return out.astype(np.float32)


if __name__ == "__main__":
    rng = np.random.default_rng(0)
    ins = {
        "x": rng.standard_normal((B, S, E), np.float32),
        "qk_w": rng.standard_normal((2 * E, E), np.float32) * 0.03,
        "qk_b": rng.standard_normal((2 * E,), np.float32) * 0.03,
        "v_w": rng.standard_normal((E, E), np.float32) * 0.03,
        "v_b": rng.standard_normal((E,), np.float32) * 0.03,
        "proj_w": rng.standard_normal((E, E), np.float32) * 0.03,
        "proj_b": rng.standard_normal((E,), np.float32) * 0.03,
        "ff_w": rng.standard_normal((E, E), np.float32) * 0.03,
        "ff_b": rng.standard_normal((E,), np.float32) * 0.03,
        "ln_g": np.ones((E,), np.float32),
        "ln_b": np.zeros((E,), np.float32),
    }
    o = kernel(**ins)
    print("ran", o.shape, o.dtype)



# revision 37
# speedup vs baseline: 1.2053x; 1.1979x over previous
"""Trainium2 Bass kernel for nn_Block_27187142983954 (dense transformer block,
per-position head-mixing attention). Data-parallel over batch: 8 cores, one
batch element each. Self-contained: hardcodes all shapes.

Per-core plan (S=4096 positions, E=1024, H=16 heads, D=64):
  - qkv projection on TensorE in fp8(e4m3) DoubleRow perf mode (K=256 per
    matmul): stationary = x feature-major fp8 tiles (host-pretransposed),
    moving = host-pretransposed fp8 weights prescaled x32 (restored at the
    PSUM->SBUF eviction); biases folded in as rank-1 (K=1) bf16 matmuls.
  - attention (per-position bilinear over heads) on VectorE in position-major
    layout with broadcast access patterns: bf16 tensor_tensor muls in 2x mode,
    partial reduction by halving-tree TT adds (2x) + final tensor_reduce.
  - softmax without max-subtraction (scores are O(1) by construction); the
    1/denominator is folded into exp(scores) BEFORE attn@v so no fp32
    broadcast-normalize is needed afterwards.
  - v is computed with host-permuted weight rows so its features land in
    (d,g) order, which keeps every broadcast AP's innermost dim contiguous.
  - proj/ff matmuls on TensorE with PE-transposed activations as stationary;
    the attn residual (x, bf16) is accumulated into the proj PSUM via an
    identity matmul so z1 needs no VectorE PSUM-read add.
  - LayerNorm stats on ScalarE via activation accum_out (Identity/Square);
    rsigma = exp(-0.5*ln(var+eps)) so softmax-exp and LN share one ACT table
    set; ln_g/ln_b of LN1 are folded into the ff weights on the host; LN2's
    affine is skipped entirely when ln_g==1 and ln_b==0 (program variant).
  - engine placement discipline: keeping ScalarE/GpSimd co-activity low
    matters more than offloading VectorE — heavy co-activity inflates every
    engine's per-op time ~20% (SBUF contention / power throttle).
"""

import sys

sys.path.insert(0, "/opt/trn_rl_repo")

import numpy as np
import ml_dtypes

E, H, DQ, DV = 1024, 16, 64, 64
B, S = 8, 4096
EPS = 1e-5
NT = S // 128  # 32 position tiles per core
BF = ml_dtypes.bfloat16
F8 = ml_dtypes.float8_e4m3
W8SCALE = 32.0  # qkv weights are ~1/32; prescale into fp8's normal range

_CACHE = {}


def _patch_tail_drain():
    """walrus in this container rejects >1 sem wait on a CTRL (Drain)
    instruction; spread the TileContext tail-drain waits over wait-nops."""
    import concourse.tile as tile
    import bass_rust
    from concourse.vector_clock import ScopedClock

    if getattr(tile.TileContext, "_drain_patched", False):
        return

    def _drain_and_barrier(self, tick_clock, wait_clock):
        nc = self.nc
        drain_inst = nc.sync.drain()
        wait_clock.add_sem_waits(
            drain_inst.ins, ScopedClock({None: tick_clock.global_clock})
        )
        si = drain_inst.ins.sync_info
        waits = list(si.on_wait) if si is not None else []
        if len(waits) > 1:
            drain_inst.ins.sync_info = bass_rust.SyncInfo(on_wait=[], on_update=[])
            for w in waits:
                nop = nc.sync.nop()
                nop.ins.sync_info = bass_rust.SyncInfo(on_wait=[w], on_update=[])
        nc.all_engine_barrier()
        assert self.sems is not None
        popped = nc._tile_sem_poison_stack.pop()
        assert popped is self._sem_poison
        nc.clear_and_free_semaphores(list(self.sems.allocated().values()))
        nc.all_engine_barrier()

    tile.TileContext._drain_and_barrier = _drain_and_barrier
    tile.TileContext._drain_patched = True


def _split_excess_waits(nc, max_on_op=1, max_on_nop=1):
    """walrus in this container rejects >2 sem waits on compute instruction
    structs and >1 on DMA/CTRL structs. Hoist excess waits onto preceding
    same-engine NOPs."""
    import concourse.mybir as mybir
    import bass_rust

    narrow = {"DMACopy", "Drain", "NoOp", "Memset", "TriggeredCopy"}
    cnt = 0
    for bb in nc.m.functions[0].blocks:
        il = bb.instructions
        out = []
        for inst in il:
            cap = 1 if inst.opcode in narrow else max_on_op
            si = inst.sync_info
            waits = list(si.on_wait) if si is not None and si.on_wait else []
            if len(waits) > cap:
                n_extra = len(waits) - cap
                extra, keep = waits[:n_extra], waits[n_extra:]
                for i0 in range(0, len(extra), max_on_nop):
                    chunk = extra[i0 : i0 + max_on_nop]
                    nop = mybir.InstNoOp(name=f"waitnop-{cnt}", ins=[], outs=[])
                    cnt += 1
                    nop.engine = inst.engine
                    nop.sync_info = bass_rust.SyncInfo(on_wait=chunk, on_update=[])
                    out.append(nop)
                inst.sync_info = bass_rust.SyncInfo(
                    on_wait=keep,
                    on_update=list(si.on_update) if si.on_update else [],
                )
            out.append(inst)
        il[:] = out


def _build_program(trivial_affine: bool, variant: int = 0):
    import concourse.bass as bass
    import concourse.tile as tile
    import concourse.mybir as mybir
    from concourse.masks import make_identity

    _patch_tail_drain()

    f32 = mybir.dt.float32
    bf16 = mybir.dt.bfloat16
    fp8 = mybir.dt.float8e4
    ALU = mybir.AluOpType
    ACT = mybir.ActivationFunctionType
    DR = mybir.MatmulPerfMode.DoubleRow

    nc = bass.Bass("TRN2", target_bir_lowering=False, debug=False, num_devices=1)

    if variant < 7:
        x_pm = nc.dram_tensor("x_pm", [S, E], f32, kind="ExternalInput").ap()
    x_bf_d = nc.dram_tensor("x_bf", [S, E], bf16, kind="ExternalInput").ap()
    if variant >= 2:
        xsum_d = nc.dram_tensor("xsum", [S, 1], f32, kind="ExternalInput").ap()
    xT = nc.dram_tensor("xT", [E, S], fp8, kind="ExternalInput").ap()
    wqkvT_d = nc.dram_tensor("wqkvT", [E, 3 * E], fp8, kind="ExternalInput").ap()
    projT_d = nc.dram_tensor("projT", [E, E], bf16, kind="ExternalInput").ap()
    ffw2T_d = nc.dram_tensor("ffw2T", [E, E], bf16, kind="ExternalInput").ap()
    bqkv_d = nc.dram_tensor("bqkv", [1, 3 * E], bf16, kind="ExternalInput").ap()
    bproj_d = nc.dram_tensor("bproj", [1, E], bf16, kind="ExternalInput").ap()
    bff2_d = nc.dram_tensor("bff2", [1, E], bf16, kind="ExternalInput").ap()
    if not trivial_affine:
        g_rep_d = nc.dram_tensor("g_rep", [128, E], f32, kind="ExternalInput").ap()
        b_rep_d = nc.dram_tensor("b_rep", [128, E], f32, kind="ExternalInput").ap()
    out_d = nc.dram_tensor("out", [S, E], f32, kind="ExternalOutput").ap()

    xT_r = xT.rearrange("(t p) s -> p t s", p=128)  # [128, 8, S]
    wqkv_r = wqkvT_d.rearrange("(t p) o -> p t o", p=128)
    proj_r = projT_d.rearrange("(t p) o -> p t o", p=128)
    ffw2_r = ffw2T_d.rearrange("(t p) o -> p t o", p=128)

    with tile.TileContext(nc) as tc:
        import contextlib

        ctx = contextlib.ExitStack()
        with ctx:
            fixed = ctx.enter_context(tc.tile_pool(name="fixed", bufs=1))
            work = ctx.enter_context(
                tc.tile_pool(name="work", bufs=(4 if variant >= 6 else 3))
            )
            work1 = ctx.enter_context(tc.tile_pool(name="work1", bufs=1))
            if variant in (10, 11):
                # qkv stage one tile ahead (REJECTED: the added concurrency
                # trips a chip-wide ~20% power/clock throttle; see v10/v11)
                qkvp = ctx.enter_context(tc.tile_pool(name="qkvp", bufs=2))
            stats = ctx.enter_context(tc.tile_pool(name="stats", bufs=8))
            psq = ctx.enter_context(
                tc.tile_pool(name="psq", bufs=(2 if variant >= 9 else 3), space="PSUM")
            )
            pst = ctx.enter_context(tc.tile_pool(name="pst", bufs=2, space="PSUM"))
            if variant >= 9:
                # 2-bank [128,1024] f32 tiles: proj and ff each evict in ONE
                # ScalarE pass (one accum read) instead of two + a pair-add
                psb = ctx.enter_context(tc.tile_pool(name="psz", bufs=2, space="PSUM"))
            else:
                psb = ctx.enter_context(
                    tc.tile_pool(
                        name="psb", bufs=(3 if variant >= 4 else 2), space="PSUM"
                    )
                )

            # ---- fixed tensors ----
            # v7: tile 0's x loads are hoisted ahead of the weight loads so
            # compute opens ~1.5MB into the DMA stream instead of ~7MB.
            wdma = nc.sync.dma_start
            pre = {}
            n_pre = 2 if variant >= 9 else 1
            bqkv_sb = fixed.tile([1, 3 * E], bf16)
            wqkv_sb = fixed.tile([128, 8, 3 * E], fp8)
            if variant >= 12:
                # startup critical path: the first qkv matmul needs ONLY
                # xf(t0) + bqkv + wqkv chunk 0 — put exactly those first
                pre_xf0 = work.tile([128, 8, 128], fp8, tag="xf")
                nc.sync.dma_start(out=pre_xf0, in_=xT_r[:, :, 0:128])
                wdma(out=bqkv_sb, in_=bqkv_d)
                for j in range(6):
                    wdma(
                        out=wqkv_sb[:, :, j * 512 : (j + 1) * 512],
                        in_=wqkv_r[:, :, j * 512 : (j + 1) * 512],
                    )
                pre_xbf0 = work.tile([128, E], bf16, tag="xbf")
                nc.sync.dma_start(out=pre_xbf0, in_=x_bf_d[0:128, :])
                pre_xs0 = work.tile([128, 1], f32, tag="xs")
                nc.sync.dma_start(out=pre_xs0, in_=xsum_d[0:128, :])
                pre[0] = (pre_xbf0, pre_xs0, pre_xf0)
                pre_xbf1 = work.tile([128, E], bf16, tag="xbf")
                nc.sync.dma_start(out=pre_xbf1, in_=x_bf_d[128:256, :])
                pre_xs1 = work.tile([128, 1], f32, tag="xs")
                nc.sync.dma_start(out=pre_xs1, in_=xsum_d[128:256, :])
                pre_xf1 = work.tile([128, 8, 128], fp8, tag="xf")
                nc.sync.dma_start(out=pre_xf1, in_=xT_r[:, :, 128:256])
                pre[1] = (pre_xbf1, pre_xs1, pre_xf1)
            else:
                if variant >= 7:
                    # first tiles' inputs first: compute can start after ~0.7MB
                    for pt_ in range(n_pre):
                        o = pt_ * 128
                        pre_xbf = work.tile([128, E], bf16, tag="xbf")
                        nc.sync.dma_start(out=pre_xbf, in_=x_bf_d[o : o + 128, :])
                        pre_xs = work.tile([128, 1], f32, tag="xs")
                        nc.sync.dma_start(out=pre_xs, in_=xsum_d[o : o + 128, :])
                        pre_xf = work.tile([128, 8, 128], fp8, tag="xf")
                        nc.sync.dma_start(out=pre_xf, in_=xT_r[:, :, o : o + 128])
                        pre[pt_] = (pre_xbf, pre_xs, pre_xf)
                wdma(out=bqkv_sb, in_=bqkv_d)
                if variant:
                    # column-group order: tile 0's first psum chunk only waits
                    # on the first 1/6th of the weight load
                    for j in range(6):
                        wdma(
                            out=wqkv_sb[:, :, j * 512 : (j + 1) * 512],
                            in_=wqkv_r[:, :, j * 512 : (j + 1) * 512],
                        )
                else:
                    for t in range(8):
                        wdma(out=wqkv_sb[:, t, :], in_=wqkv_r[:, t, :])
            # v10: proj/ff weights ride the (otherwise idle) GpSimd SWDGE
            # queue so ~18 trigger slots don't serialize the sync queue at
            # startup in front of the per-tile x loads
            w2dma = wdma
            proj_sb = fixed.tile([128, 8, E], bf16)
            ffw2_sb = fixed.tile([128, 8, E], bf16)
            for t in range(8):
                w2dma(out=proj_sb[:, t, :], in_=proj_r[:, t, :])
            for t in range(8):
                w2dma(out=ffw2_sb[:, t, :], in_=ffw2_r[:, t, :])
            bproj_sb = fixed.tile([1, E], bf16)
            w2dma(out=bproj_sb, in_=bproj_d)
            bff2_sb = fixed.tile([1, E], bf16)
            w2dma(out=bff2_sb, in_=bff2_d)
            if not trivial_affine:
                g_rep = fixed.tile([128, E], f32)
                nc.sync.dma_start(out=g_rep, in_=g_rep_d)
                b_rep = fixed.tile([128, E], f32)
                nc.sync.dma_start(out=b_rep, in_=b_rep_d)
            ones_row = fixed.tile([1, 128], bf16)
            nc.vector.memset(ones_row, 1.0)
            ident = fixed.tile([128, 128], bf16)
            make_identity(nc, ident)
            eps_sb = fixed.tile([128, 1], f32)
            nc.vector.memset(eps_sb, EPS)

            inv_n = 1.0 / float(E)

            def emit_tail(gl_p, xp_p, s0_p, s1z2=None):
                """Deferred tile tail: z2 = gelu_out + x, LN2, affine, store.
                Emitted one iteration late so the DVE z2/LN ops land in the
                next tile's exp-wait window instead of stalling on gelu."""
                if variant >= 9:
                    # gl_p is the ff PSUM [128,1024] (2 banks); the gelu
                    # eviction itself is deferred to here so it lands right
                    # after the next tile's softmax exp in ScalarE's FIFO
                    # (the gelu ACT-table swap then sits off-critical too).
                    # s1z2 arrives as the xs tile: sum(z2) = sum(gelu) + sum(x).
                    gl = work.tile([128, E], bf16, tag="gl")
                    sgl = stats.tile([128, 1], f32, tag="sgl")
                    nc.scalar.activation(gl, gl_p, ACT.Gelu, accum_out=sgl)
                    s1t = stats.tile([128, 1], f32, tag="s1z2")
                    nc.scalar.activation(s1t, sgl, ACT.Identity, bias=s1z2)
                    s1z2 = s1t
                    gl_p = gl
                if variant >= 7:
                    # bf16 residual add: 2x DVE mode, and the f32 x load is
                    # dropped entirely (xp_p is the bf16 x tile here)
                    z2 = work.tile([128, E], bf16, tag="z2")
                    nc.vector.tensor_tensor(z2, gl_p, xp_p, ALU.add)
                else:
                    z2 = work.tile([128, E], f32, tag="z2")
                    for j in range(2):
                        nc.vector.tensor_tensor(
                            z2[:, j * 512 : (j + 1) * 512],
                            gl_p[:, j * 512 : (j + 1) * 512],
                            xp_p[:, j * 512 : (j + 1) * 512],
                            ALU.add,
                        )
                rs2 = stats.tile([128, 1], f32, tag="rs2")
                mrs2 = stats.tile([128, 1], f32, tag="mrs2")
                lnscr2 = work1.tile([128, E], bf16, tag="lnscr2")
                layer_norm(z2, rs2, mrs2, lnscr2, s1_pre=s1z2)
                if trivial_affine:
                    out_t = work.tile([128, E], f32, tag="out_t")
                    nc.scalar.activation(
                        out_t, z2, ACT.Identity, bias=mrs2, scale=rs2
                    )
                else:
                    zn = work1.tile([128, E], f32, tag="zn")
                    nc.scalar.activation(zn, z2, ACT.Identity, bias=mrs2, scale=rs2)
                    zn2 = work1.tile([128, E], f32, tag="zn2")
                    nc.gpsimd.tensor_tensor(zn2, zn, g_rep, ALU.mult)
                    out_t = work.tile([128, E], f32, tag="out_t")
                    nc.gpsimd.tensor_tensor(out_t, zn2, b_rep, ALU.add)
                nc.sync.dma_start(out=out_d[s0_p : s0_p + 128, :], in_=out_t)

            pending = None

            def layer_norm_scalar(s1, s2, rs_out, mrs_out):
                """LN scalar chain entirely on ScalarE: keeps every [P,1] op
                out of DVE's in-order queue so DVE never head-of-line blocks
                on cross-engine stats. rsigma = exp(-0.5*ln(s2/N + eps - mu^2))."""
                mneg = stats.tile([128, 1], f32, tag="mneg")
                nc.scalar.activation(mneg, s1, ACT.Identity, scale=-inv_n)
                mu2 = stats.tile([128, 1], f32, tag="mu2")
                nc.scalar.activation(mu2, s1, ACT.Square, scale=inv_n)
                emm = stats.tile([128, 1], f32, tag="emm")
                nc.scalar.activation(emm, mu2, ACT.Identity, scale=-1.0, bias=eps_sb)
                lnv = stats.tile([128, 1], f32, tag="lnv")
                nc.scalar.activation(lnv, s2, ACT.Ln, scale=inv_n, bias=emm)
                nc.scalar.activation(rs_out, lnv, ACT.Exp, scale=-0.5)
                nc.scalar.activation(mrs_out, mneg, ACT.Identity, scale=rs_out)

            def layer_norm(z, rs_out, mrs_out, scratch_bf, s1_pre=None, s2_pre=None):
                """Compute rsigma and -mu*rsigma of z [128, E] (fp32).
                If s1_pre/s2_pre are given (sum / sum-of-squares precomputed,
                e.g. fused into the PSUM eviction), those passes are skipped."""
                if s1_pre is None:
                    s1 = stats.tile([128, 1], f32, tag="s1")
                    nc.scalar.activation(scratch_bf, z, ACT.Identity, accum_out=s1)
                else:
                    s1 = s1_pre
                if s2_pre is None:
                    s2 = stats.tile([128, 1], f32, tag="s2")
                    nc.scalar.activation(scratch_bf, z, ACT.Square, accum_out=s2)
                else:
                    s2 = s2_pre
                if variant >= 8:
                    layer_norm_scalar(s1, s2, rs_out, mrs_out)
                    return
                mu = stats.tile([128, 1], f32, tag="mu")
                lnv = stats.tile([128, 1], f32, tag="lnv")
                nc.vector.tensor_scalar_mul(mu, s1, inv_n)
                mu2 = stats.tile([128, 1], f32, tag="mu2")
                nc.vector.tensor_tensor(mu2, mu, mu, ALU.mult)
                var = stats.tile([128, 1], f32, tag="var")
                nc.vector.scalar_tensor_tensor(
                    var, in0=s2, scalar=inv_n, in1=mu2, op0=ALU.mult, op1=ALU.subtract
                )
                nc.scalar.activation(lnv, var, ACT.Ln, bias=eps_sb)
                nc.scalar.activation(rs_out, lnv, ACT.Exp, scale=-0.5)
                nc.vector.scalar_tensor_tensor(
                    mrs_out, in0=mu, scalar=-1.0, in1=rs_out, op0=ALU.mult, op1=ALU.mult
                )

            def qkv_stage(t):
                """x loads + qkv projection for tile t; returns (xbf, xs, qkv_sb).
                For variant>=10 this runs one tile AHEAD of the main body."""
                s0 = t * 128
                if t in pre:
                    xbf, xs, xf = pre[t]
                else:
                    xbf = work.tile([128, E], bf16, tag="xbf")
                    nc.sync.dma_start(out=xbf, in_=x_bf_d[s0 : s0 + 128, :])
                    xs = work.tile([128, 1], f32, tag="xs")
                    nc.sync.dma_start(out=xs, in_=xsum_d[s0 : s0 + 128, :])
                    xf = work.tile([128, 8, 128], fp8, tag="xf")
                    nc.sync.dma_start(out=xf, in_=xT_r[:, :, s0 : s0 + 128])
                qkv_sb = qkvp.tile([128, 3 * E], bf16, tag="qkv")
                for j in range(6):
                    ps = psq.tile([128, 512], f32, tag="psq")
                    for e2 in range(4):
                        nc.tensor.matmul(
                            ps,
                            xf[:, 2 * e2 : 2 * e2 + 2, :],
                            wqkv_sb[:, 2 * e2 : 2 * e2 + 2, j * 512 : (j + 1) * 512],
                            start=(e2 == 0),
                            stop=False,
                            perf_mode=DR,
                        )
                    nc.tensor.matmul(
                        ps,
                        ones_row,
                        bqkv_sb[:, j * 512 : (j + 1) * 512],
                        start=False,
                        stop=True,
                    )
                    nc.scalar.mul(qkv_sb[:, j * 512 : (j + 1) * 512], ps, 1.0 / W8SCALE)
                return xbf, xs, qkv_sb

            cur = None
            for t in range(NT):
                s0 = t * 128
                if variant in (10, 11):
                    xp = None
                    if t == 0:
                        cur = qkv_stage(0)
                    xbf, xs, qkv_sb = cur
                elif variant >= 7:
                    xp = None
                    if t in pre:
                        xbf, xs, xf = pre[t]
                    else:
                        xbf = work.tile([128, E], bf16, tag="xbf")
                        nc.sync.dma_start(out=xbf, in_=x_bf_d[s0 : s0 + 128, :])
                        xs = work.tile([128, 1], f32, tag="xs")
                        nc.sync.dma_start(out=xs, in_=xsum_d[s0 : s0 + 128, :])
                        xf = work.tile([128, 8, 128], fp8, tag="xf")
                        nc.sync.dma_start(out=xf, in_=xT_r[:, :, s0 : s0 + 128])
                else:
                    xp = work.tile([128, E], f32, tag="xp")
                    nc.sync.dma_start(out=xp, in_=x_pm[s0 : s0 + 128, :])
                    xbf = work.tile([128, E], bf16, tag="xbf")
                    nc.sync.dma_start(out=xbf, in_=x_bf_d[s0 : s0 + 128, :])
                    if variant >= 2:
                        xs = work.tile([128, 1], f32, tag="xs")
                        nc.sync.dma_start(out=xs, in_=xsum_d[s0 : s0 + 128, :])
                    xf = work.tile([128, 8, 128], fp8, tag="xf")
                    nc.sync.dma_start(out=xf, in_=xT_r[:, :, s0 : s0 + 128])

                if variant not in (10, 11):
                    # ---- qkv projection (fp8 DoubleRow, weights prescaled x32) ----
                    qkv_sb = work1.tile([128, 3 * E], bf16, tag="qkv")
                    for j in range(6):
                        ps = psq.tile([128, 512], f32, tag="psq")
                        for e2 in range(4):
                            nc.tensor.matmul(
                                ps,
                                xf[:, 2 * e2 : 2 * e2 + 2, :],
                                wqkv_sb[:, 2 * e2 : 2 * e2 + 2, j * 512 : (j + 1) * 512],
                                start=(e2 == 0),
                                stop=False,
                                perf_mode=DR,
                            )
                        nc.tensor.matmul(
                            ps,
                            ones_row,
                            bqkv_sb[:, j * 512 : (j + 1) * 512],
                            start=False,
                            stop=True,
                        )
                        nc.scalar.mul(
                            qkv_sb[:, j * 512 : (j + 1) * 512], ps, 1.0 / W8SCALE
                        )

                q3 = qkv_sb[:, 0:E].rearrange("p (h d) -> p h d", h=H)
                k3 = qkv_sb[:, E : 2 * E].rearrange("p (g d) -> p g d", g=H)
                v3 = qkv_sb[:, 2 * E : 3 * E].rearrange("p (d g) -> p d g", d=DV)

                # ---- QK^T scores ----
                prod = work1.tile([128, 8, 16, 64], bf16, tag="prod")
                scr = work1.tile([128, 8192], bf16, tag="scr")
                scores = work.tile([128, H, H], bf16 if variant >= 8 else f32, tag="scores")
                p_sb = work.tile([128, H, H], bf16, tag="p_sb")
                for half in range(2):
                    h0 = half * 8
                    qb = q3[:, h0 : h0 + 8, :].unsqueeze(2).broadcast_to([128, 8, 16, 64])
                    kb = k3.unsqueeze(1).broadcast_to([128, 8, 16, 64])
                    nc.vector.tensor_tensor(prod, kb, qb, ALU.mult)
                    t1 = scr[:, 0:4096].rearrange("p (a g d) -> p a g d", a=8, g=16)
                    nc.vector.tensor_tensor(
                        t1, prod[:, :, :, 0:32], prod[:, :, :, 32:64], ALU.add
                    )
                    t2 = scr[:, 4096:6144].rearrange("p (a g d) -> p a g d", a=8, g=16)
                    nc.vector.tensor_tensor(
                        t2, t1[:, :, :, 0:16], t1[:, :, :, 16:32], ALU.add
                    )
                    t3 = scr[:, 6144:7168].rearrange("p (a g d) -> p a g d", a=8, g=16)
                    nc.vector.tensor_tensor(
                        t3, t2[:, :, :, 0:8], t2[:, :, :, 8:16], ALU.add
                    )
                    t4 = scr[:, 7168:7680].rearrange("p (a g d) -> p a g d", a=8, g=16)
                    nc.vector.tensor_tensor(
                        t4, t3[:, :, :, 0:4], t3[:, :, :, 4:8], ALU.add
                    )
                    if variant >= 8:
                        # finish with 2x-mode TT halvings instead of a 1x reduce
                        t5 = scr[:, 7680:7936].rearrange(
                            "p (a g d) -> p a g d", a=8, g=16
                        )
                        nc.vector.tensor_tensor(
                            t5, t4[:, :, :, 0:2], t4[:, :, :, 2:4], ALU.add
                        )
                        nc.vector.tensor_tensor(
                            scores[:, h0 : h0 + 8, :].unsqueeze(3),
                            t5[:, :, :, 0:1],
                            t5[:, :, :, 1:2],
                            ALU.add,
                        )
                    else:
                        nc.vector.tensor_reduce(
                            scores[:, h0 : h0 + 8, :],
                            t4,
                            axis=mybir.AxisListType.X,
                            op=ALU.add,
                        )

                # ---- softmax (no max-subtraction; fold 1/den into p before AV) ----
                nc.scalar.activation(p_sb, scores, ACT.Exp)
                if pending is not None and variant < 9:
                    emit_tail(*pending)
                    pending = None
                den = stats.tile([128, H], f32, tag="den")
                nc.vector.tensor_reduce(
                    den, p_sb, axis=mybir.AxisListType.X, op=ALU.add
                )
                rden = stats.tile([128, H], f32, tag="rden")
                nc.vector.reciprocal(rden, den)
                p_nm = work.tile([128, H, H], bf16, tag="p_nm")
                nc.vector.tensor_tensor(
                    p_nm,
                    p_sb,
                    rden.unsqueeze(2).broadcast_to([128, H, H]),
                    ALU.mult,
                )
                if pending is not None:
                    # v9: tail emitted after den/rden/p_nm so DVE's in-order
                    # queue hits den (waits only on exp) before z2 (waits on
                    # the deferred gelu eviction)
                    emit_tail(*pending)
                    pending = None
                if variant in (10, 11) and t + 1 < NT:
                    # next tile's qkv stage here: its ScalarE evictions land
                    # ahead of this tile's LN/eviction stream, so the next
                    # tile's first prod has its inputs a full tile early
                    cur = qkv_stage(t + 1)

                # ---- attn @ v ----
                attn_bf = work.tile([128, E], bf16, tag="attn_bf")
                a3 = attn_bf.rearrange("p (h d) -> p h d", h=H)
                prod_flat = prod.rearrange("p a g d -> p (a g d)")
                for half in range(2):
                    h0 = half * 8
                    # reuse prod's memory with a contiguous [128, 8, 64, 16] layout
                    pa = prod_flat.rearrange("p (a d g) -> p a d g", a=8, d=DV)
                    pb = (
                        p_nm[:, h0 : h0 + 8, :]
                        .unsqueeze(2)
                        .broadcast_to([128, 8, 64, 16])
                    )
                    vb = v3.unsqueeze(1).broadcast_to([128, 8, 64, 16])
                    nc.vector.tensor_tensor(pa, vb, pb, ALU.mult)
                    u1 = scr[:, 0:4096].rearrange("p (a d g) -> p a d g", a=8, d=64)
                    nc.vector.tensor_tensor(
                        u1, pa[:, :, :, 0:8], pa[:, :, :, 8:16], ALU.add
                    )
                    u2 = scr[:, 4096:6144].rearrange("p (a d g) -> p a d g", a=8, d=64)
                    nc.vector.tensor_tensor(
                        u2, u1[:, :, :, 0:4], u1[:, :, :, 4:8], ALU.add
                    )
                    u3 = scr[:, 6144:7168].rearrange("p (a d g) -> p a d g", a=8, d=64)
                    nc.vector.tensor_tensor(
                        u3, u2[:, :, :, 0:2], u2[:, :, :, 2:4], ALU.add
                    )
                    nc.vector.tensor_tensor(
                        a3[:, h0 : h0 + 8, :].unsqueeze(3),
                        u3[:, :, :, 0:1],
                        u3[:, :, :, 1:2],
                        ALU.add,
                    )

                # ---- transpose attn_out to feature-major (4 or 8 per PSUM tile) ----
                # v7: PSUM->SBUF evictions on ScalarE (DVE is the bottleneck)
                evict = nc.scalar.copy if variant >= 7 else nc.vector.tensor_copy
                attn_fm = work.tile([128, 8, 128], bf16, tag="attn_fm")
                TG = 8 if variant >= 5 else 4
                for q in range(8 // TG):
                    pt = pst.tile([128, TG * 128], bf16, tag="pst")
                    for e4 in range(TG):
                        e = q * TG + e4
                        nc.tensor.transpose(
                            pt[:, e4 * 128 : (e4 + 1) * 128],
                            attn_bf[:, e * 128 : (e + 1) * 128],
                            ident,
                        )
                    evict(
                        attn_fm[:, q * TG : (q + 1) * TG, :].rearrange(
                            "p a b -> p (a b)"
                        ),
                        pt,
                    )

                # ---- proj + residual (x folded in via identity matmul) ----
                z1 = work1.tile([128, E], f32, tag="z1")
                lnscr = work1.tile([128, E], bf16, tag="lnscr")
                s1parts = []
                s2parts = []
                ps2w = None
                for j in range(2):
                    if variant >= 9:
                        if j == 0:
                            ps2w = psb.tile([128, 1024], f32, tag="psz")
                        ps2 = ps2w[:, j * 512 : (j + 1) * 512]
                    else:
                        ps2 = psb.tile([128, 512], f32, tag="psb")
                    for e in range(8):
                        nc.tensor.matmul(
                            ps2,
                            attn_fm[:, e, :],
                            proj_sb[:, e, j * 512 : (j + 1) * 512],
                            start=(e == 0),
                            stop=False,
                        )
                    nc.tensor.matmul(
                        ps2,
                        ones_row,
                        bproj_sb[:, j * 512 : (j + 1) * 512],
                        start=False,
                        stop=False,
                    )
                    nc.tensor.matmul(
                        ps2,
                        ident,
                        xbf[:, j * 512 : (j + 1) * 512],
                        start=False,
                        stop=True,
                    )
                    if variant >= 9:
                        continue
                    if variant:
                        # fuse the sum-of-z1 accumulation into the eviction
                        s1p = stats.tile([128, 1], f32, tag=f"s1p{j}")
                        s1parts.append(s1p)
                        nc.scalar.activation(
                            z1[:, j * 512 : (j + 1) * 512],
                            ps2,
                            ACT.Identity,
                            accum_out=s1p,
                        )
                        if variant >= 3:
                            # sum-of-squares per chunk straight from PSUM too
                            s2p = stats.tile([128, 1], f32, tag=f"s2p{j}")
                            s2parts.append(s2p)
                            nc.scalar.activation(
                                lnscr[:, j * 512 : (j + 1) * 512],
                                ps2,
                                ACT.Square,
                                accum_out=s2p,
                            )
                    else:
                        nc.scalar.copy(z1[:, j * 512 : (j + 1) * 512], ps2)

                # ---- LN1 (g,b folded into ff weights) ----
                rs1 = stats.tile([128, 1], f32, tag="rs1")
                mrs1 = stats.tile([128, 1], f32, tag="mrs1")
                if variant >= 9:
                    # single [128,1024] eviction across both PSUM banks: one
                    # accum read per stat, no pair-adds
                    s1p = stats.tile([128, 1], f32, tag="s1p")
                    nc.scalar.activation(z1, ps2w, ACT.Identity, accum_out=s1p)
                    s2p = stats.tile([128, 1], f32, tag="s2p")
                    nc.scalar.activation(lnscr, ps2w, ACT.Square, accum_out=s2p)
                    layer_norm(z1, rs1, mrs1, lnscr, s1_pre=s1p, s2_pre=s2p)
                elif variant:
                    s1f = stats.tile([128, 1], f32, tag="s1f")
                    if variant >= 7:
                        nc.scalar.activation(
                            s1f, s1parts[0], ACT.Identity, bias=s1parts[1]
                        )
                    else:
                        nc.vector.tensor_tensor(s1f, s1parts[0], s1parts[1], ALU.add)
                    if variant >= 3:
                        s2f = stats.tile([128, 1], f32, tag="s2f")
                        if variant >= 7:
                            nc.scalar.activation(
                                s2f, s2parts[0], ACT.Identity, bias=s2parts[1]
                            )
                        else:
                            nc.vector.tensor_tensor(
                                s2f, s2parts[0], s2parts[1], ALU.add
                            )
                        layer_norm(z1, rs1, mrs1, lnscr, s1_pre=s1f, s2_pre=s2f)
                    else:
                        layer_norm(z1, rs1, mrs1, lnscr, s1_pre=s1f)
                else:
                    layer_norm(z1, rs1, mrs1, lnscr)
                ln1_bf = work.tile([128, E], bf16, tag="ln1_bf")
                nc.scalar.activation(ln1_bf, z1, ACT.Identity, bias=mrs1, scale=rs1)

                ln1_fm = work.tile([128, 8, 128], bf16, tag="ln1_fm")
                for q in range(8 // TG):
                    pt = pst.tile([128, TG * 128], bf16, tag="pst")
                    for e4 in range(TG):
                        e = q * TG + e4
                        nc.tensor.transpose(
                            pt[:, e4 * 128 : (e4 + 1) * 128],
                            ln1_bf[:, e * 128 : (e + 1) * 128],
                            ident,
                        )
                    evict(
                        ln1_fm[:, q * TG : (q + 1) * TG, :].rearrange(
                            "p a b -> p (a b)"
                        ),
                        pt,
                    )

                # ---- ff + gelu (z2/LN2 deferred to next iteration's tail) ----
                if variant >= 9:
                    # matmuls only; the gelu eviction is deferred into the
                    # next iteration's tail (after its softmax exp)
                    ps3w = psb.tile([128, 1024], f32, tag="psz")
                    for j in range(2):
                        pj = ps3w[:, j * 512 : (j + 1) * 512]
                        for e in range(8):
                            nc.tensor.matmul(
                                pj,
                                ln1_fm[:, e, :],
                                ffw2_sb[:, e, j * 512 : (j + 1) * 512],
                                start=(e == 0),
                                stop=False,
                            )
                        nc.tensor.matmul(
                            pj,
                            ones_row,
                            bff2_sb[:, j * 512 : (j + 1) * 512],
                            start=False,
                            stop=True,
                        )
                    pending = (ps3w, xbf, s0, xs)
                    continue
                gl = work.tile([128, E], bf16 if variant >= 7 else f32, tag="gl")
                sglparts = []
                for j in range(2):
                    ps3 = psb.tile([128, 512], f32, tag="psb")
                    for e in range(8):
                        nc.tensor.matmul(
                            ps3,
                            ln1_fm[:, e, :],
                            ffw2_sb[:, e, j * 512 : (j + 1) * 512],
                            start=(e == 0),
                            stop=False,
                        )
                    nc.tensor.matmul(
                        ps3,
                        ones_row,
                        bff2_sb[:, j * 512 : (j + 1) * 512],
                        start=False,
                        stop=True,
                    )
                    if variant >= 2:
                        # fuse the sum-of-gelu accumulation into the eviction;
                        # sum(z2) = sum(gelu) + sum(x) (host-precomputed xsum)
                        sgl = stats.tile([128, 1], f32, tag=f"sgl{j}")
                        sglparts.append(sgl)
                        nc.scalar.activation(
                            gl[:, j * 512 : (j + 1) * 512],
                            ps3,
                            ACT.Gelu,
                            accum_out=sgl,
                        )
                    else:
                        nc.scalar.activation(
                            gl[:, j * 512 : (j + 1) * 512], ps3, ACT.Gelu
                        )

                if variant == 8:
                    # prefetch the exp/ln ACT table now (gelu swapped it out),
                    # so the next tile's softmax exp isn't stuck behind a
                    # 1.3us ACT_TABLE_LOAD on the critical path
                    atld = stats.tile([128, 1], f32, tag="atld")
                    nc.scalar.activation(atld, eps_sb, ACT.Ln)
                    atld2 = stats.tile([128, 1], f32, tag="atld2")
                    nc.scalar.activation(atld2, eps_sb, ACT.Exp)

                if variant >= 7:
                    # [P,1] pair-adds ride ScalarE (bias is added pre-func)
                    sgf = stats.tile([128, 1], f32, tag="sgf")
                    nc.scalar.activation(
                        sgf, sglparts[0], ACT.Identity, bias=sglparts[1]
                    )
                    s1z2 = stats.tile([128, 1], f32, tag="s1z2")
                    nc.scalar.activation(s1z2, sgf, ACT.Identity, bias=xs)
                    pending = (gl, xbf, s0, s1z2)
                elif variant >= 2:
                    sgf = stats.tile([128, 1], f32, tag="sgf")
                    nc.vector.tensor_tensor(sgf, sglparts[0], sglparts[1], ALU.add)
                    s1z2 = stats.tile([128, 1], f32, tag="s1z2")
                    nc.vector.tensor_tensor(s1z2, sgf, xs, ALU.add)
                    pending = (gl, xp, s0, s1z2)
                else:
                    pending = (gl, xp, s0)

            emit_tail(*pending)

    _split_excess_waits(nc)
    return nc


def _host_prep(inputs, trivial_affine=None):
    x = np.asarray(inputs["x"], np.float32)
    qk_w = np.asarray(inputs["qk_w"], np.float32)
    qk_b = np.asarray(inputs["qk_b"], np.float32)
    v_w = np.asarray(inputs["v_w"], np.float32)
    v_b = np.asarray(inputs["v_b"], np.float32)
    proj_w = np.asarray(inputs["proj_w"], np.float32)
    proj_b = np.asarray(inputs["proj_b"], np.float32)
    ff_w = np.asarray(inputs["ff_w"], np.float32)
    ff_b = np.asarray(inputs["ff_b"], np.float32)
    ln_g = np.asarray(inputs["ln_g"], np.float32)
    ln_b = np.asarray(inputs["ln_b"], np.float32)

    if trivial_affine is None:
        trivial_affine = bool(
            np.allclose(ln_g, 1.0, atol=1e-7) and np.allclose(ln_b, 0.0, atol=1e-7)
        )

    scale = 1.0 / np.sqrt(DQ).astype(np.float32)
    Wq = qk_w[:E] * scale
    bq = qk_b[:E] * scale
    Wk = qk_w[E:]
    bk = qk_b[E:]
    g_idx, d_idx = np.meshgrid(np.arange(H), np.arange(DV), indexing="ij")
    perm = np.empty(E, np.int64)
    perm[(d_idx * H + g_idx).ravel()] = (g_idx * DV + d_idx).ravel()
    Wv2 = v_w[perm]
    bv2 = v_b[perm]

    wqkvT = np.ascontiguousarray(
        (np.concatenate([Wq, Wk, Wv2], 0) * W8SCALE).T.astype(F8)
    )  # [E, 3E] fp8, prescaled
    bqkv = (np.concatenate([bq, bk, bv2]) * W8SCALE)[None, :].astype(BF)  # [1, 3E]
    projT = np.ascontiguousarray(proj_w.T.astype(BF))  # [E, E]
    bproj = proj_b[None, :].astype(BF)
    ffw2T = np.ascontiguousarray((ff_w * ln_g[None, :]).T.astype(BF))
    bff2 = (ff_b + ff_w @ ln_b)[None, :].astype(BF)

    shared = {
        "wqkvT": wqkvT,
        "bqkv": bqkv,
        "projT": projT,
        "bproj": bproj,
        "ffw2T": ffw2T,
        "bff2": bff2,
    }
    if not trivial_affine:
        shared["g_rep"] = np.ascontiguousarray(
            np.broadcast_to(ln_g[None, :], (128, E)), np.float32
        )
        shared["b_rep"] = np.ascontiguousarray(
            np.broadcast_to(ln_b[None, :], (128, E)), np.float32
        )
    in_maps = []
    for b in range(B):
        xb = np.ascontiguousarray(x[b])  # [S, E] f32
        xTb = np.ascontiguousarray(xb.T.astype(F8))  # [E, S] fp8
        m = {
            "x_pm": xb,
            "x_bf": xb.astype(BF),
            "xT": xTb,
            "xsum": np.ascontiguousarray(xb.sum(-1, dtype=np.float32)[:, None]),
        }
        m.update(shared)
        in_maps.append(m)
    return in_maps


def kernel(**inputs) -> np.ndarray:
    from concourse.bass_utils import run_bass_kernel_spmd

    trivial_affine = bool(
        np.allclose(np.asarray(inputs["ln_g"]), 1.0, atol=1e-7)
        and np.allclose(np.asarray(inputs["ln_b"]), 0.0, atol=1e-7)
    )
    variant = 12  # v9 scheduling + startup DMA reorder (xf first)
    key = ("nc", trivial_affine, variant)
    if key not in _CACHE:
        _CACHE[key] = _build_program(trivial_affine, variant)
    nc = _CACHE[key]

    in_maps = _host_prep(inputs, trivial_affine)
    res = run_bass_kernel_spmd(nc, in_maps, core_ids=list(range(B)))
    out = np.stack([res.results[b]["out"] for b in range(B)], 0)
    return out.astype(np.float32)


if __name__ == "__main__":
    rng = np.random.default_rng(0)
    ins = {
        "x": rng.standard_normal((B, S, E), np.float32),
        "qk_w": rng.standard_normal((2 * E, E), np.float32) * 0.03,
        "qk_b": rng.standard_normal((2 * E,), np.float32) * 0.03,
        "v_w": rng.standard_normal((E, E), np.float32) * 0.03,
        "v_b": rng.standard_normal((E,), np.float32) * 0.03,
        "proj_w": rng.standard_normal((E, E), np.float32) * 0.03,
        "proj_b": rng.standard_normal((E,), np.float32) * 0.03,
        "ff_w": rng.standard_normal((E, E), np.float32) * 0.03,
        "ff_b": rng.standard_normal((E,), np.float32) * 0.03,
        "ln_g": np.ones((E,), np.float32),
        "ln_b": np.zeros((E,), np.float32),
    }
    o = kernel(**ins)
    print("ran", o.shape, o.dtype)



# revision 39
# speedup vs baseline: 1.2084x; 1.0026x over previous
"""Trainium2 Bass kernel for nn_Block_27187142983954 (dense transformer block,
per-position head-mixing attention). Data-parallel over batch: 8 cores, one
batch element each. Self-contained: hardcodes all shapes.

Per-core plan (S=4096 positions, E=1024, H=16 heads, D=64):
  - qkv projection on TensorE in fp8(e4m3) DoubleRow perf mode (K=256 per
    matmul): stationary = x feature-major fp8 tiles (host-pretransposed),
    moving = host-pretransposed fp8 weights prescaled x32 (restored at the
    PSUM->SBUF eviction); biases folded in as rank-1 (K=1) bf16 matmuls.
  - attention (per-position bilinear over heads) on VectorE in position-major
    layout with broadcast access patterns: bf16 tensor_tensor muls in 2x mode,
    partial reduction by halving-tree TT adds (2x) + final tensor_reduce.
  - softmax without max-subtraction (scores are O(1) by construction); the
    1/denominator is folded into exp(scores) BEFORE attn@v so no fp32
    broadcast-normalize is needed afterwards.
  - v is computed with host-permuted weight rows so its features land in
    (d,g) order, which keeps every broadcast AP's innermost dim contiguous.
  - proj/ff matmuls on TensorE with PE-transposed activations as stationary;
    the attn residual (x, bf16) is accumulated into the proj PSUM via an
    identity matmul so z1 needs no VectorE PSUM-read add.
  - LayerNorm stats on ScalarE via activation accum_out (Identity/Square);
    rsigma = exp(-0.5*ln(var+eps)) so softmax-exp and LN share one ACT table
    set; ln_g/ln_b of LN1 are folded into the ff weights on the host; LN2's
    affine is skipped entirely when ln_g==1 and ln_b==0 (program variant).
  - engine placement discipline: keeping ScalarE/GpSimd co-activity low
    matters more than offloading VectorE — heavy co-activity inflates every
    engine's per-op time ~20% (SBUF contention / power throttle).

v12 schedule notes (measured on HW, 1489us -> 1369us):
  - VectorE is the bottleneck (~89% busy; the per-position attention is
    irreducibly elementwise: TT=2x max, reduce/pool/custom-DVE ops are 1x,
    so the 2x TT halving tree is already at the DVE read-port roofline).
    Everything else must stay OFF the DVE in-order queue: PSUM evictions,
    LN scalar chains, and [P,1] stat adds all run on ScalarE (activation
    scale/bias tricks), so DVE never head-of-line blocks on cross-engine.
  - proj/ff PSUM targets are single [128,1024] 2-bank tiles: one eviction
    activation (+accum) instead of two + a pair-add.
  - The gelu eviction is deferred into the NEXT tile after its softmax exp:
    the gelu ACT-table swap (1.3us) then sits off the exp->den critical
    path, and exp is 2nd in ScalarE's per-tile FIFO.
  - DO NOT software-pipeline the qkv stage a tile ahead (v10/v11): the
    extra sustained concurrency trips a chip-wide ~20% clock/power throttle
    (every engine's op durations scale by exactly 1.2x, uniformly over the
    whole run). Same for SWDGE weight loads. Keep total co-activity low.
  - GpSimd compute offload is useless while DVE runs TT ops: they share an
    exclusive SBUF port pair (the loser fully blocks).
  - Startup: the first qkv matmul needs only xf(t0)+bqkv+wqkv[j0]; those
    DMAs go first on the sync queue.
"""

import sys

sys.path.insert(0, "/opt/trn_rl_repo")

import numpy as np
import ml_dtypes

E, H, DQ, DV = 1024, 16, 64, 64
B, S = 8, 4096
EPS = 1e-5
NT = S // 128  # 32 position tiles per core
BF = ml_dtypes.bfloat16
F8 = ml_dtypes.float8_e4m3
W8SCALE = 32.0  # qkv weights are ~1/32; prescale into fp8's normal range

_CACHE = {}


def _patch_tail_drain():
    """walrus in this container rejects >1 sem wait on a CTRL (Drain)
    instruction; spread the TileContext tail-drain waits over wait-nops."""
    import concourse.tile as tile
    import bass_rust
    from concourse.vector_clock import ScopedClock

    if getattr(tile.TileContext, "_drain_patched", False):
        return

    def _drain_and_barrier(self, tick_clock, wait_clock):
        nc = self.nc
        drain_inst = nc.sync.drain()
        wait_clock.add_sem_waits(
            drain_inst.ins, ScopedClock({None: tick_clock.global_clock})
        )
        si = drain_inst.ins.sync_info
        waits = list(si.on_wait) if si is not None else []
        if len(waits) > 1:
            drain_inst.ins.sync_info = bass_rust.SyncInfo(on_wait=[], on_update=[])
            for w in waits:
                nop = nc.sync.nop()
                nop.ins.sync_info = bass_rust.SyncInfo(on_wait=[w], on_update=[])
        nc.all_engine_barrier()
        assert self.sems is not None
        popped = nc._tile_sem_poison_stack.pop()
        assert popped is self._sem_poison
        nc.clear_and_free_semaphores(list(self.sems.allocated().values()))
        nc.all_engine_barrier()

    tile.TileContext._drain_and_barrier = _drain_and_barrier
    tile.TileContext._drain_patched = True


def _split_excess_waits(nc, max_on_op=1, max_on_nop=1):
    """walrus in this container rejects >2 sem waits on compute instruction
    structs and >1 on DMA/CTRL structs. Hoist excess waits onto preceding
    same-engine NOPs."""
    import concourse.mybir as mybir
    import bass_rust

    narrow = {"DMACopy", "Drain", "NoOp", "Memset", "TriggeredCopy"}
    cnt = 0
    for bb in nc.m.functions[0].blocks:
        il = bb.instructions
        out = []
        for inst in il:
            cap = 1 if inst.opcode in narrow else max_on_op
            si = inst.sync_info
            waits = list(si.on_wait) if si is not None and si.on_wait else []
            if len(waits) > cap:
                n_extra = len(waits) - cap
                extra, keep = waits[:n_extra], waits[n_extra:]
                for i0 in range(0, len(extra), max_on_nop):
                    chunk = extra[i0 : i0 + max_on_nop]
                    nop = mybir.InstNoOp(name=f"waitnop-{cnt}", ins=[], outs=[])
                    cnt += 1
                    nop.engine = inst.engine
                    nop.sync_info = bass_rust.SyncInfo(on_wait=chunk, on_update=[])
                    out.append(nop)
                inst.sync_info = bass_rust.SyncInfo(
                    on_wait=keep,
                    on_update=list(si.on_update) if si.on_update else [],
                )
            out.append(inst)
        il[:] = out


def _build_program(trivial_affine: bool, variant: int = 0):
    import concourse.bass as bass
    import concourse.tile as tile
    import concourse.mybir as mybir
    from concourse.masks import make_identity

    _patch_tail_drain()

    f32 = mybir.dt.float32
    bf16 = mybir.dt.bfloat16
    fp8 = mybir.dt.float8e4
    ALU = mybir.AluOpType
    ACT = mybir.ActivationFunctionType
    DR = mybir.MatmulPerfMode.DoubleRow

    nc = bass.Bass("TRN2", target_bir_lowering=False, debug=False, num_devices=1)

    if variant < 7:
        x_pm = nc.dram_tensor("x_pm", [S, E], f32, kind="ExternalInput").ap()
    x_bf_d = nc.dram_tensor("x_bf", [S, E], bf16, kind="ExternalInput").ap()
    if variant >= 2:
        xsum_d = nc.dram_tensor("xsum", [S, 1], f32, kind="ExternalInput").ap()
    xT = nc.dram_tensor("xT", [E, S], fp8, kind="ExternalInput").ap()
    wqkvT_d = nc.dram_tensor("wqkvT", [E, 3 * E], fp8, kind="ExternalInput").ap()
    projT_d = nc.dram_tensor("projT", [E, E], bf16, kind="ExternalInput").ap()
    ffw2T_d = nc.dram_tensor("ffw2T", [E, E], bf16, kind="ExternalInput").ap()
    bqkv_d = nc.dram_tensor("bqkv", [1, 3 * E], bf16, kind="ExternalInput").ap()
    bproj_d = nc.dram_tensor("bproj", [1, E], bf16, kind="ExternalInput").ap()
    bff2_d = nc.dram_tensor("bff2", [1, E], bf16, kind="ExternalInput").ap()
    if not trivial_affine:
        g_rep_d = nc.dram_tensor("g_rep", [128, E], f32, kind="ExternalInput").ap()
        b_rep_d = nc.dram_tensor("b_rep", [128, E], f32, kind="ExternalInput").ap()
    out_d = nc.dram_tensor("out", [S, E], f32, kind="ExternalOutput").ap()

    xT_r = xT.rearrange("(t p) s -> p t s", p=128)  # [128, 8, S]
    wqkv_r = wqkvT_d.rearrange("(t p) o -> p t o", p=128)
    proj_r = projT_d.rearrange("(t p) o -> p t o", p=128)
    ffw2_r = ffw2T_d.rearrange("(t p) o -> p t o", p=128)

    with tile.TileContext(nc) as tc:
        import contextlib

        ctx = contextlib.ExitStack()
        with ctx:
            fixed = ctx.enter_context(tc.tile_pool(name="fixed", bufs=1))
            work = ctx.enter_context(
                tc.tile_pool(name="work", bufs=(4 if variant >= 6 else 3))
            )
            work1 = ctx.enter_context(tc.tile_pool(name="work1", bufs=1))
            if variant in (10, 11, 13):
                # v10/11: qkv a FULL tile ahead (REJECTED: filling the PE-idle
                # window trips a chip-wide ~20% clock throttle). v13: a HALF
                # step — qkv(t+1) emitted just before ff(t), extending the
                # existing PE-busy stretch instead of the idle window.
                qkvp = ctx.enter_context(tc.tile_pool(name="qkvp", bufs=2))
            stats = ctx.enter_context(tc.tile_pool(name="stats", bufs=8))
            psq = ctx.enter_context(
                tc.tile_pool(name="psq", bufs=(2 if variant >= 9 else 3), space="PSUM")
            )
            pst = ctx.enter_context(tc.tile_pool(name="pst", bufs=2, space="PSUM"))
            if variant >= 9:
                # 2-bank [128,1024] f32 tiles: proj and ff each evict in ONE
                # ScalarE pass (one accum read) instead of two + a pair-add
                psb = ctx.enter_context(tc.tile_pool(name="psz", bufs=2, space="PSUM"))
            else:
                psb = ctx.enter_context(
                    tc.tile_pool(
                        name="psb", bufs=(3 if variant >= 4 else 2), space="PSUM"
                    )
                )

            # ---- fixed tensors ----
            # v7: tile 0's x loads are hoisted ahead of the weight loads so
            # compute opens ~1.5MB into the DMA stream instead of ~7MB.
            wdma = nc.sync.dma_start
            pre = {}
            n_pre = 2 if variant >= 9 else 1
            bqkv_sb = fixed.tile([1, 3 * E], bf16)
            wqkv_sb = fixed.tile([128, 8, 3 * E], fp8)
            if variant >= 12:
                # startup critical path: the first qkv matmul needs ONLY
                # xf(t0) + bqkv + wqkv chunk 0 — put exactly those first
                pre_xf0 = work.tile([128, 8, 128], fp8, tag="xf")
                nc.sync.dma_start(out=pre_xf0, in_=xT_r[:, :, 0:128])
                wdma(out=bqkv_sb, in_=bqkv_d)
                for j in range(6):
                    wdma(
                        out=wqkv_sb[:, :, j * 512 : (j + 1) * 512],
                        in_=wqkv_r[:, :, j * 512 : (j + 1) * 512],
                    )
                pre_xbf0 = work.tile([128, E], bf16, tag="xbf")
                nc.sync.dma_start(out=pre_xbf0, in_=x_bf_d[0:128, :])
                pre_xs0 = work.tile([128, 1], f32, tag="xs")
                nc.sync.dma_start(out=pre_xs0, in_=xsum_d[0:128, :])
                pre[0] = (pre_xbf0, pre_xs0, pre_xf0)
                pre_xbf1 = work.tile([128, E], bf16, tag="xbf")
                nc.sync.dma_start(out=pre_xbf1, in_=x_bf_d[128:256, :])
                pre_xs1 = work.tile([128, 1], f32, tag="xs")
                nc.sync.dma_start(out=pre_xs1, in_=xsum_d[128:256, :])
                pre_xf1 = work.tile([128, 8, 128], fp8, tag="xf")
                nc.sync.dma_start(out=pre_xf1, in_=xT_r[:, :, 128:256])
                pre[1] = (pre_xbf1, pre_xs1, pre_xf1)
            else:
                if variant >= 7:
                    # first tiles' inputs first: compute can start after ~0.7MB
                    for pt_ in range(n_pre):
                        o = pt_ * 128
                        pre_xbf = work.tile([128, E], bf16, tag="xbf")
                        nc.sync.dma_start(out=pre_xbf, in_=x_bf_d[o : o + 128, :])
                        pre_xs = work.tile([128, 1], f32, tag="xs")
                        nc.sync.dma_start(out=pre_xs, in_=xsum_d[o : o + 128, :])
                        pre_xf = work.tile([128, 8, 128], fp8, tag="xf")
                        nc.sync.dma_start(out=pre_xf, in_=xT_r[:, :, o : o + 128])
                        pre[pt_] = (pre_xbf, pre_xs, pre_xf)
                wdma(out=bqkv_sb, in_=bqkv_d)
                if variant:
                    # column-group order: tile 0's first psum chunk only waits
                    # on the first 1/6th of the weight load
                    for j in range(6):
                        wdma(
                            out=wqkv_sb[:, :, j * 512 : (j + 1) * 512],
                            in_=wqkv_r[:, :, j * 512 : (j + 1) * 512],
                        )
                else:
                    for t in range(8):
                        wdma(out=wqkv_sb[:, t, :], in_=wqkv_r[:, t, :])
            # v10: proj/ff weights ride the (otherwise idle) GpSimd SWDGE
            # queue so ~18 trigger slots don't serialize the sync queue at
            # startup in front of the per-tile x loads
            w2dma = wdma
            proj_sb = fixed.tile([128, 8, E], bf16)
            ffw2_sb = fixed.tile([128, 8, E], bf16)
            for t in range(8):
                w2dma(out=proj_sb[:, t, :], in_=proj_r[:, t, :])
            for t in range(8):
                w2dma(out=ffw2_sb[:, t, :], in_=ffw2_r[:, t, :])
            bproj_sb = fixed.tile([1, E], bf16)
            w2dma(out=bproj_sb, in_=bproj_d)
            bff2_sb = fixed.tile([1, E], bf16)
            w2dma(out=bff2_sb, in_=bff2_d)
            if not trivial_affine:
                g_rep = fixed.tile([128, E], f32)
                nc.sync.dma_start(out=g_rep, in_=g_rep_d)
                b_rep = fixed.tile([128, E], f32)
                nc.sync.dma_start(out=b_rep, in_=b_rep_d)
            ones_row = fixed.tile([1, 128], bf16)
            nc.vector.memset(ones_row, 1.0)
            ident = fixed.tile([128, 128], bf16)
            make_identity(nc, ident)
            eps_sb = fixed.tile([128, 1], f32)
            nc.vector.memset(eps_sb, EPS)

            inv_n = 1.0 / float(E)

            def emit_tail(gl_p, xp_p, s0_p, s1z2=None):
                """Deferred tile tail: z2 = gelu_out + x, LN2, affine, store.
                Emitted one iteration late so the DVE z2/LN ops land in the
                next tile's exp-wait window instead of stalling on gelu."""
                if variant >= 9:
                    # gl_p is the ff PSUM [128,1024] (2 banks); the gelu
                    # eviction itself is deferred to here so it lands right
                    # after the next tile's softmax exp in ScalarE's FIFO
                    # (the gelu ACT-table swap then sits off-critical too).
                    # s1z2 arrives as the xs tile: sum(z2) = sum(gelu) + sum(x).
                    gl = work.tile([128, E], bf16, tag="gl")
                    sgl = stats.tile([128, 1], f32, tag="sgl")
                    nc.scalar.activation(gl, gl_p, ACT.Gelu, accum_out=sgl)
                    s1t = stats.tile([128, 1], f32, tag="s1z2")
                    nc.scalar.activation(s1t, sgl, ACT.Identity, bias=s1z2)
                    s1z2 = s1t
                    gl_p = gl
                if variant >= 7:
                    # bf16 residual add: 2x DVE mode, and the f32 x load is
                    # dropped entirely (xp_p is the bf16 x tile here)
                    z2 = work.tile([128, E], bf16, tag="z2")
                    nc.vector.tensor_tensor(z2, gl_p, xp_p, ALU.add)
                else:
                    z2 = work.tile([128, E], f32, tag="z2")
                    for j in range(2):
                        nc.vector.tensor_tensor(
                            z2[:, j * 512 : (j + 1) * 512],
                            gl_p[:, j * 512 : (j + 1) * 512],
                            xp_p[:, j * 512 : (j + 1) * 512],
                            ALU.add,
                        )
                rs2 = stats.tile([128, 1], f32, tag="rs2")
                mrs2 = stats.tile([128, 1], f32, tag="mrs2")
                lnscr2 = work1.tile([128, E], bf16, tag="lnscr2")
                layer_norm(z2, rs2, mrs2, lnscr2, s1_pre=s1z2)
                if trivial_affine:
                    out_t = work.tile([128, E], f32, tag="out_t")
                    nc.scalar.activation(
                        out_t, z2, ACT.Identity, bias=mrs2, scale=rs2
                    )
                else:
                    zn = work1.tile([128, E], f32, tag="zn")
                    nc.scalar.activation(zn, z2, ACT.Identity, bias=mrs2, scale=rs2)
                    zn2 = work1.tile([128, E], f32, tag="zn2")
                    nc.gpsimd.tensor_tensor(zn2, zn, g_rep, ALU.mult)
                    out_t = work.tile([128, E], f32, tag="out_t")
                    nc.gpsimd.tensor_tensor(out_t, zn2, b_rep, ALU.add)
                nc.sync.dma_start(out=out_d[s0_p : s0_p + 128, :], in_=out_t)

            pending = None

            def layer_norm_scalar(s1, s2, rs_out, mrs_out):
                """LN scalar chain entirely on ScalarE: keeps every [P,1] op
                out of DVE's in-order queue so DVE never head-of-line blocks
                on cross-engine stats. rsigma = exp(-0.5*ln(s2/N + eps - mu^2))."""
                mneg = stats.tile([128, 1], f32, tag="mneg")
                nc.scalar.activation(mneg, s1, ACT.Identity, scale=-inv_n)
                mu2 = stats.tile([128, 1], f32, tag="mu2")
                nc.scalar.activation(mu2, s1, ACT.Square, scale=inv_n)
                emm = stats.tile([128, 1], f32, tag="emm")
                nc.scalar.activation(emm, mu2, ACT.Identity, scale=-1.0, bias=eps_sb)
                lnv = stats.tile([128, 1], f32, tag="lnv")
                nc.scalar.activation(lnv, s2, ACT.Ln, scale=inv_n, bias=emm)
                nc.scalar.activation(rs_out, lnv, ACT.Exp, scale=-0.5)
                nc.scalar.activation(mrs_out, mneg, ACT.Identity, scale=rs_out)

            def layer_norm(z, rs_out, mrs_out, scratch_bf, s1_pre=None, s2_pre=None):
                """Compute rsigma and -mu*rsigma of z [128, E] (fp32).
                If s1_pre/s2_pre are given (sum / sum-of-squares precomputed,
                e.g. fused into the PSUM eviction), those passes are skipped."""
                if s1_pre is None:
                    s1 = stats.tile([128, 1], f32, tag="s1")
                    nc.scalar.activation(scratch_bf, z, ACT.Identity, accum_out=s1)
                else:
                    s1 = s1_pre
                if s2_pre is None:
                    s2 = stats.tile([128, 1], f32, tag="s2")
                    nc.scalar.activation(scratch_bf, z, ACT.Square, accum_out=s2)
                else:
                    s2 = s2_pre
                if variant >= 8:
                    layer_norm_scalar(s1, s2, rs_out, mrs_out)
                    return
                mu = stats.tile([128, 1], f32, tag="mu")
                lnv = stats.tile([128, 1], f32, tag="lnv")
                nc.vector.tensor_scalar_mul(mu, s1, inv_n)
                mu2 = stats.tile([128, 1], f32, tag="mu2")
                nc.vector.tensor_tensor(mu2, mu, mu, ALU.mult)
                var = stats.tile([128, 1], f32, tag="var")
                nc.vector.scalar_tensor_tensor(
                    var, in0=s2, scalar=inv_n, in1=mu2, op0=ALU.mult, op1=ALU.subtract
                )
                nc.scalar.activation(lnv, var, ACT.Ln, bias=eps_sb)
                nc.scalar.activation(rs_out, lnv, ACT.Exp, scale=-0.5)
                nc.vector.scalar_tensor_tensor(
                    mrs_out, in0=mu, scalar=-1.0, in1=rs_out, op0=ALU.mult, op1=ALU.mult
                )

            def qkv_stage(t):
                """x loads + qkv projection for tile t; returns (xbf, xs, qkv_sb).
                For variant>=10 this runs one tile AHEAD of the main body."""
                s0 = t * 128
                if t in pre:
                    xbf, xs, xf = pre[t]
                else:
                    xbf = work.tile([128, E], bf16, tag="xbf")
                    nc.sync.dma_start(out=xbf, in_=x_bf_d[s0 : s0 + 128, :])
                    xs = work.tile([128, 1], f32, tag="xs")
                    nc.sync.dma_start(out=xs, in_=xsum_d[s0 : s0 + 128, :])
                    xf = work.tile([128, 8, 128], fp8, tag="xf")
                    nc.sync.dma_start(out=xf, in_=xT_r[:, :, s0 : s0 + 128])
                qkv_sb = qkvp.tile([128, 3 * E], bf16, tag="qkv")
                for j in range(6):
                    ps = psq.tile([128, 512], f32, tag="psq")
                    for e2 in range(4):
                        nc.tensor.matmul(
                            ps,
                            xf[:, 2 * e2 : 2 * e2 + 2, :],
                            wqkv_sb[:, 2 * e2 : 2 * e2 + 2, j * 512 : (j + 1) * 512],
                            start=(e2 == 0),
                            stop=False,
                            perf_mode=DR,
                        )
                    nc.tensor.matmul(
                        ps,
                        ones_row,
                        bqkv_sb[:, j * 512 : (j + 1) * 512],
                        start=False,
                        stop=True,
                    )
                    nc.scalar.mul(qkv_sb[:, j * 512 : (j + 1) * 512], ps, 1.0 / W8SCALE)
                return xbf, xs, qkv_sb

            cur = None
            for t in range(NT):
                s0 = t * 128
                if variant in (10, 11, 13):
                    xp = None
                    if t == 0:
                        cur = qkv_stage(0)
                    xbf, xs, qkv_sb = cur
                elif variant >= 7:
                    xp = None
                    if t in pre:
                        xbf, xs, xf = pre[t]
                    else:
                        xbf = work.tile([128, E], bf16, tag="xbf")
                        nc.sync.dma_start(out=xbf, in_=x_bf_d[s0 : s0 + 128, :])
                        xs = work.tile([128, 1], f32, tag="xs")
                        nc.sync.dma_start(out=xs, in_=xsum_d[s0 : s0 + 128, :])
                        xf = work.tile([128, 8, 128], fp8, tag="xf")
                        nc.sync.dma_start(out=xf, in_=xT_r[:, :, s0 : s0 + 128])
                else:
                    xp = work.tile([128, E], f32, tag="xp")
                    nc.sync.dma_start(out=xp, in_=x_pm[s0 : s0 + 128, :])
                    xbf = work.tile([128, E], bf16, tag="xbf")
                    nc.sync.dma_start(out=xbf, in_=x_bf_d[s0 : s0 + 128, :])
                    if variant >= 2:
                        xs = work.tile([128, 1], f32, tag="xs")
                        nc.sync.dma_start(out=xs, in_=xsum_d[s0 : s0 + 128, :])
                    xf = work.tile([128, 8, 128], fp8, tag="xf")
                    nc.sync.dma_start(out=xf, in_=xT_r[:, :, s0 : s0 + 128])

                if variant not in (10, 11, 13):
                    # ---- qkv projection (fp8 DoubleRow, weights prescaled x32) ----
                    qkv_sb = work1.tile([128, 3 * E], bf16, tag="qkv")
                    for j in range(6):
                        ps = psq.tile([128, 512], f32, tag="psq")
                        for e2 in range(4):
                            nc.tensor.matmul(
                                ps,
                                xf[:, 2 * e2 : 2 * e2 + 2, :],
                                wqkv_sb[:, 2 * e2 : 2 * e2 + 2, j * 512 : (j + 1) * 512],
                                start=(e2 == 0),
                                stop=False,
                                perf_mode=DR,
                            )
                        nc.tensor.matmul(
                            ps,
                            ones_row,
                            bqkv_sb[:, j * 512 : (j + 1) * 512],
                            start=False,
                            stop=True,
                        )
                        nc.scalar.mul(
                            qkv_sb[:, j * 512 : (j + 1) * 512], ps, 1.0 / W8SCALE
                        )

                q3 = qkv_sb[:, 0:E].rearrange("p (h d) -> p h d", h=H)
                k3 = qkv_sb[:, E : 2 * E].rearrange("p (g d) -> p g d", g=H)
                v3 = qkv_sb[:, 2 * E : 3 * E].rearrange("p (d g) -> p d g", d=DV)

                # ---- QK^T scores ----
                prod = work1.tile([128, 8, 16, 64], bf16, tag="prod")
                scr = work1.tile([128, 8192], bf16, tag="scr")
                scores = work.tile([128, H, H], bf16 if variant >= 8 else f32, tag="scores")
                p_sb = work.tile([128, H, H], bf16, tag="p_sb")
                for half in range(2):
                    h0 = half * 8
                    qb = q3[:, h0 : h0 + 8, :].unsqueeze(2).broadcast_to([128, 8, 16, 64])
                    kb = k3.unsqueeze(1).broadcast_to([128, 8, 16, 64])
                    nc.vector.tensor_tensor(prod, kb, qb, ALU.mult)
                    t1 = scr[:, 0:4096].rearrange("p (a g d) -> p a g d", a=8, g=16)
                    nc.vector.tensor_tensor(
                        t1, prod[:, :, :, 0:32], prod[:, :, :, 32:64], ALU.add
                    )
                    t2 = scr[:, 4096:6144].rearrange("p (a g d) -> p a g d", a=8, g=16)
                    nc.vector.tensor_tensor(
                        t2, t1[:, :, :, 0:16], t1[:, :, :, 16:32], ALU.add
                    )
                    t3 = scr[:, 6144:7168].rearrange("p (a g d) -> p a g d", a=8, g=16)
                    nc.vector.tensor_tensor(
                        t3, t2[:, :, :, 0:8], t2[:, :, :, 8:16], ALU.add
                    )
                    t4 = scr[:, 7168:7680].rearrange("p (a g d) -> p a g d", a=8, g=16)
                    nc.vector.tensor_tensor(
                        t4, t3[:, :, :, 0:4], t3[:, :, :, 4:8], ALU.add
                    )
                    if variant >= 8:
                        # finish with 2x-mode TT halvings instead of a 1x reduce
                        t5 = scr[:, 7680:7936].rearrange(
                            "p (a g d) -> p a g d", a=8, g=16
                        )
                        nc.vector.tensor_tensor(
                            t5, t4[:, :, :, 0:2], t4[:, :, :, 2:4], ALU.add
                        )
                        nc.vector.tensor_tensor(
                            scores[:, h0 : h0 + 8, :].unsqueeze(3),
                            t5[:, :, :, 0:1],
                            t5[:, :, :, 1:2],
                            ALU.add,
                        )
                    else:
                        nc.vector.tensor_reduce(
                            scores[:, h0 : h0 + 8, :],
                            t4,
                            axis=mybir.AxisListType.X,
                            op=ALU.add,
                        )

                # ---- softmax (no max-subtraction; fold 1/den into p before AV) ----
                nc.scalar.activation(p_sb, scores, ACT.Exp)
                if pending is not None and variant < 9:
                    emit_tail(*pending)
                    pending = None
                den = stats.tile([128, H], f32, tag="den")
                nc.vector.tensor_reduce(
                    den, p_sb, axis=mybir.AxisListType.X, op=ALU.add
                )
                rden = stats.tile([128, H], f32, tag="rden")
                nc.vector.reciprocal(rden, den)
                p_nm = work.tile([128, H, H], bf16, tag="p_nm")
                nc.vector.tensor_tensor(
                    p_nm,
                    p_sb,
                    rden.unsqueeze(2).broadcast_to([128, H, H]),
                    ALU.mult,
                )
                if pending is not None:
                    # v9: tail emitted after den/rden/p_nm so DVE's in-order
                    # queue hits den (waits only on exp) before z2 (waits on
                    # the deferred gelu eviction)
                    emit_tail(*pending)
                    pending = None
                if variant in (10, 11) and t + 1 < NT:
                    # next tile's qkv stage here: its ScalarE evictions land
                    # ahead of this tile's LN/eviction stream, so the next
                    # tile's first prod has its inputs a full tile early
                    cur = qkv_stage(t + 1)

                # ---- attn @ v ----
                attn_bf = work.tile([128, E], bf16, tag="attn_bf")
                a3 = attn_bf.rearrange("p (h d) -> p h d", h=H)
                prod_flat = prod.rearrange("p a g d -> p (a g d)")
                for half in range(2):
                    h0 = half * 8
                    # reuse prod's memory with a contiguous [128, 8, 64, 16] layout
                    pa = prod_flat.rearrange("p (a d g) -> p a d g", a=8, d=DV)
                    pb = (
                        p_nm[:, h0 : h0 + 8, :]
                        .unsqueeze(2)
                        .broadcast_to([128, 8, 64, 16])
                    )
                    vb = v3.unsqueeze(1).broadcast_to([128, 8, 64, 16])
                    nc.vector.tensor_tensor(pa, vb, pb, ALU.mult)
                    u1 = scr[:, 0:4096].rearrange("p (a d g) -> p a d g", a=8, d=64)
                    nc.vector.tensor_tensor(
                        u1, pa[:, :, :, 0:8], pa[:, :, :, 8:16], ALU.add
                    )
                    u2 = scr[:, 4096:6144].rearrange("p (a d g) -> p a d g", a=8, d=64)
                    nc.vector.tensor_tensor(
                        u2, u1[:, :, :, 0:4], u1[:, :, :, 4:8], ALU.add
                    )
                    u3 = scr[:, 6144:7168].rearrange("p (a d g) -> p a d g", a=8, d=64)
                    nc.vector.tensor_tensor(
                        u3, u2[:, :, :, 0:2], u2[:, :, :, 2:4], ALU.add
                    )
                    nc.vector.tensor_tensor(
                        a3[:, h0 : h0 + 8, :].unsqueeze(3),
                        u3[:, :, :, 0:1],
                        u3[:, :, :, 1:2],
                        ALU.add,
                    )

                # ---- transpose attn_out to feature-major (4 or 8 per PSUM tile) ----
                # v7: PSUM->SBUF evictions on ScalarE (DVE is the bottleneck)
                evict = nc.scalar.copy if variant >= 7 else nc.vector.tensor_copy
                attn_fm = work.tile([128, 8, 128], bf16, tag="attn_fm")
                TG = 8 if variant >= 5 else 4
                for q in range(8 // TG):
                    pt = pst.tile([128, TG * 128], bf16, tag="pst")
                    for e4 in range(TG):
                        e = q * TG + e4
                        nc.tensor.transpose(
                            pt[:, e4 * 128 : (e4 + 1) * 128],
                            attn_bf[:, e * 128 : (e + 1) * 128],
                            ident,
                        )
                    evict(
                        attn_fm[:, q * TG : (q + 1) * TG, :].rearrange(
                            "p a b -> p (a b)"
                        ),
                        pt,
                    )

                # ---- proj + residual (x folded in via identity matmul) ----
                z1 = work1.tile([128, E], f32, tag="z1")
                lnscr = work1.tile([128, E], bf16, tag="lnscr")
                s1parts = []
                s2parts = []
                ps2w = None
                for j in range(2):
                    if variant >= 9:
                        if j == 0:
                            ps2w = psb.tile([128, 1024], f32, tag="psz")
                        ps2 = ps2w[:, j * 512 : (j + 1) * 512]
                    else:
                        ps2 = psb.tile([128, 512], f32, tag="psb")
                    for e in range(8):
                        nc.tensor.matmul(
                            ps2,
                            attn_fm[:, e, :],
                            proj_sb[:, e, j * 512 : (j + 1) * 512],
                            start=(e == 0),
                            stop=False,
                        )
                    nc.tensor.matmul(
                        ps2,
                        ones_row,
                        bproj_sb[:, j * 512 : (j + 1) * 512],
                        start=False,
                        stop=False,
                    )
                    nc.tensor.matmul(
                        ps2,
                        ident,
                        xbf[:, j * 512 : (j + 1) * 512],
                        start=False,
                        stop=True,
                    )
                    if variant >= 9:
                        continue
                    if variant:
                        # fuse the sum-of-z1 accumulation into the eviction
                        s1p = stats.tile([128, 1], f32, tag=f"s1p{j}")
                        s1parts.append(s1p)
                        nc.scalar.activation(
                            z1[:, j * 512 : (j + 1) * 512],
                            ps2,
                            ACT.Identity,
                            accum_out=s1p,
                        )
                        if variant >= 3:
                            # sum-of-squares per chunk straight from PSUM too
                            s2p = stats.tile([128, 1], f32, tag=f"s2p{j}")
                            s2parts.append(s2p)
                            nc.scalar.activation(
                                lnscr[:, j * 512 : (j + 1) * 512],
                                ps2,
                                ACT.Square,
                                accum_out=s2p,
                            )
                    else:
                        nc.scalar.copy(z1[:, j * 512 : (j + 1) * 512], ps2)

                # ---- LN1 (g,b folded into ff weights) ----
                rs1 = stats.tile([128, 1], f32, tag="rs1")
                mrs1 = stats.tile([128, 1], f32, tag="mrs1")
                if variant >= 9:
                    # single [128,1024] eviction across both PSUM banks: one
                    # accum read per stat, no pair-adds
                    s1p = stats.tile([128, 1], f32, tag="s1p")
                    nc.scalar.activation(z1, ps2w, ACT.Identity, accum_out=s1p)
                    s2p = stats.tile([128, 1], f32, tag="s2p")
                    nc.scalar.activation(lnscr, ps2w, ACT.Square, accum_out=s2p)
                    layer_norm(z1, rs1, mrs1, lnscr, s1_pre=s1p, s2_pre=s2p)
                elif variant:
                    s1f = stats.tile([128, 1], f32, tag="s1f")
                    if variant >= 7:
                        nc.scalar.activation(
                            s1f, s1parts[0], ACT.Identity, bias=s1parts[1]
                        )
                    else:
                        nc.vector.tensor_tensor(s1f, s1parts[0], s1parts[1], ALU.add)
                    if variant >= 3:
                        s2f = stats.tile([128, 1], f32, tag="s2f")
                        if variant >= 7:
                            nc.scalar.activation(
                                s2f, s2parts[0], ACT.Identity, bias=s2parts[1]
                            )
                        else:
                            nc.vector.tensor_tensor(
                                s2f, s2parts[0], s2parts[1], ALU.add
                            )
                        layer_norm(z1, rs1, mrs1, lnscr, s1_pre=s1f, s2_pre=s2f)
                    else:
                        layer_norm(z1, rs1, mrs1, lnscr, s1_pre=s1f)
                else:
                    layer_norm(z1, rs1, mrs1, lnscr)
                ln1_bf = work.tile([128, E], bf16, tag="ln1_bf")
                nc.scalar.activation(ln1_bf, z1, ACT.Identity, bias=mrs1, scale=rs1)

                ln1_fm = work.tile([128, 8, 128], bf16, tag="ln1_fm")
                for q in range(8 // TG):
                    pt = pst.tile([128, TG * 128], bf16, tag="pst")
                    for e4 in range(TG):
                        e = q * TG + e4
                        nc.tensor.transpose(
                            pt[:, e4 * 128 : (e4 + 1) * 128],
                            ln1_bf[:, e * 128 : (e + 1) * 128],
                            ident,
                        )
                    evict(
                        ln1_fm[:, q * TG : (q + 1) * TG, :].rearrange(
                            "p a b -> p (a b)"
                        ),
                        pt,
                    )

                # ---- ff + gelu (z2/LN2 deferred to next iteration's tail) ----
                if variant >= 13 and t + 1 < NT:
                    # half-step qkv prefetch: lands after ln1 in the TensorE
                    # FIFO and after ln1-evictions in ScalarE's, so the next
                    # tile's first prod input is ready without filling the
                    # PE-idle window (throttle-safe, unlike v10/v11)
                    cur = qkv_stage(t + 1)
                if variant >= 9:
                    # matmuls only; the gelu eviction is deferred into the
                    # next iteration's tail (after its softmax exp)
                    ps3w = psb.tile([128, 1024], f32, tag="psz")
                    for j in range(2):
                        pj = ps3w[:, j * 512 : (j + 1) * 512]
                        for e in range(8):
                            nc.tensor.matmul(
                                pj,
                                ln1_fm[:, e, :],
                                ffw2_sb[:, e, j * 512 : (j + 1) * 512],
                                start=(e == 0),
                                stop=False,
                            )
                        nc.tensor.matmul(
                            pj,
                            ones_row,
                            bff2_sb[:, j * 512 : (j + 1) * 512],
                            start=False,
                            stop=True,
                        )
                    pending = (ps3w, xbf, s0, xs)
                    continue
                gl = work.tile([128, E], bf16 if variant >= 7 else f32, tag="gl")
                sglparts = []
                for j in range(2):
                    ps3 = psb.tile([128, 512], f32, tag="psb")
                    for e in range(8):
                        nc.tensor.matmul(
                            ps3,
                            ln1_fm[:, e, :],
                            ffw2_sb[:, e, j * 512 : (j + 1) * 512],
                            start=(e == 0),
                            stop=False,
                        )
                    nc.tensor.matmul(
                        ps3,
                        ones_row,
                        bff2_sb[:, j * 512 : (j + 1) * 512],
                        start=False,
                        stop=True,
                    )
                    if variant >= 2:
                        # fuse the sum-of-gelu accumulation into the eviction;
                        # sum(z2) = sum(gelu) + sum(x) (host-precomputed xsum)
                        sgl = stats.tile([128, 1], f32, tag=f"sgl{j}")
                        sglparts.append(sgl)
                        nc.scalar.activation(
                            gl[:, j * 512 : (j + 1) * 512],
                            ps3,
                            ACT.Gelu,
                            accum_out=sgl,
                        )
                    else:
                        nc.scalar.activation(
                            gl[:, j * 512 : (j + 1) * 512], ps3, ACT.Gelu
                        )

                if variant == 8:
                    # prefetch the exp/ln ACT table now (gelu swapped it out),
                    # so the next tile's softmax exp isn't stuck behind a
                    # 1.3us ACT_TABLE_LOAD on the critical path
                    atld = stats.tile([128, 1], f32, tag="atld")
                    nc.scalar.activation(atld, eps_sb, ACT.Ln)
                    atld2 = stats.tile([128, 1], f32, tag="atld2")
                    nc.scalar.activation(atld2, eps_sb, ACT.Exp)

                if variant >= 7:
                    # [P,1] pair-adds ride ScalarE (bias is added pre-func)
                    sgf = stats.tile([128, 1], f32, tag="sgf")
                    nc.scalar.activation(
                        sgf, sglparts[0], ACT.Identity, bias=sglparts[1]
                    )
                    s1z2 = stats.tile([128, 1], f32, tag="s1z2")
                    nc.scalar.activation(s1z2, sgf, ACT.Identity, bias=xs)
                    pending = (gl, xbf, s0, s1z2)
                elif variant >= 2:
                    sgf = stats.tile([128, 1], f32, tag="sgf")
                    nc.vector.tensor_tensor(sgf, sglparts[0], sglparts[1], ALU.add)
                    s1z2 = stats.tile([128, 1], f32, tag="s1z2")
                    nc.vector.tensor_tensor(s1z2, sgf, xs, ALU.add)
                    pending = (gl, xp, s0, s1z2)
                else:
                    pending = (gl, xp, s0)

            emit_tail(*pending)

    _split_excess_waits(nc)
    return nc


def _host_prep(inputs, trivial_affine=None):
    x = np.asarray(inputs["x"], np.float32)
    qk_w = np.asarray(inputs["qk_w"], np.float32)
    qk_b = np.asarray(inputs["qk_b"], np.float32)
    v_w = np.asarray(inputs["v_w"], np.float32)
    v_b = np.asarray(inputs["v_b"], np.float32)
    proj_w = np.asarray(inputs["proj_w"], np.float32)
    proj_b = np.asarray(inputs["proj_b"], np.float32)
    ff_w = np.asarray(inputs["ff_w"], np.float32)
    ff_b = np.asarray(inputs["ff_b"], np.float32)
    ln_g = np.asarray(inputs["ln_g"], np.float32)
    ln_b = np.asarray(inputs["ln_b"], np.float32)

    if trivial_affine is None:
        trivial_affine = bool(
            np.allclose(ln_g, 1.0, atol=1e-7) and np.allclose(ln_b, 0.0, atol=1e-7)
        )

    scale = 1.0 / np.sqrt(DQ).astype(np.float32)
    Wq = qk_w[:E] * scale
    bq = qk_b[:E] * scale
    Wk = qk_w[E:]
    bk = qk_b[E:]
    g_idx, d_idx = np.meshgrid(np.arange(H), np.arange(DV), indexing="ij")
    perm = np.empty(E, np.int64)
    perm[(d_idx * H + g_idx).ravel()] = (g_idx * DV + d_idx).ravel()
    Wv2 = v_w[perm]
    bv2 = v_b[perm]

    wqkvT = np.ascontiguousarray(
        (np.concatenate([Wq, Wk, Wv2], 0) * W8SCALE).T.astype(F8)
    )  # [E, 3E] fp8, prescaled
    bqkv = (np.concatenate([bq, bk, bv2]) * W8SCALE)[None, :].astype(BF)  # [1, 3E]
    projT = np.ascontiguousarray(proj_w.T.astype(BF))  # [E, E]
    bproj = proj_b[None, :].astype(BF)
    ffw2T = np.ascontiguousarray((ff_w * ln_g[None, :]).T.astype(BF))
    bff2 = (ff_b + ff_w @ ln_b)[None, :].astype(BF)

    shared = {
        "wqkvT": wqkvT,
        "bqkv": bqkv,
        "projT": projT,
        "bproj": bproj,
        "ffw2T": ffw2T,
        "bff2": bff2,
    }
    if not trivial_affine:
        shared["g_rep"] = np.ascontiguousarray(
            np.broadcast_to(ln_g[None, :], (128, E)), np.float32
        )
        shared["b_rep"] = np.ascontiguousarray(
            np.broadcast_to(ln_b[None, :], (128, E)), np.float32
        )
    in_maps = []
    for b in range(B):
        xb = np.ascontiguousarray(x[b])  # [S, E] f32
        xTb = np.ascontiguousarray(xb.T.astype(F8))  # [E, S] fp8
        m = {
            "x_pm": xb,
            "x_bf": xb.astype(BF),
            "xT": xTb,
            "xsum": np.ascontiguousarray(xb.sum(-1, dtype=np.float32)[:, None]),
        }
        m.update(shared)
        in_maps.append(m)
    return in_maps


def kernel(**inputs) -> np.ndarray:
    from concourse.bass_utils import run_bass_kernel_spmd

    trivial_affine = bool(
        np.allclose(np.asarray(inputs["ln_g"]), 1.0, atol=1e-7)
        and np.allclose(np.asarray(inputs["ln_b"]), 0.0, atol=1e-7)
    )
    variant = 13  # v12 + half-step qkv prefetch before ff
    key = ("nc", trivial_affine, variant)
    if key not in _CACHE:
        _CACHE[key] = _build_program(trivial_affine, variant)
    nc = _CACHE[key]

    in_maps = _host_prep(inputs, trivial_affine)
    res = run_bass_kernel_spmd(nc, in_maps, core_ids=list(range(B)))
    out = np.stack([res.results[b]["out"] for b in range(B)], 0)
    return out.astype(np.float32)


if __name__ == "__main__":
    rng = np.random.default_rng(0)
    ins = {
        "x": rng.standard_normal((B, S, E), np.float32),
        "qk_w": rng.standard_normal((2 * E, E), np.float32) * 0.03,
        "qk_b": rng.standard_normal((2 * E,), np.float32) * 0.03,
        "v_w": rng.standard_normal((E, E), np.float32) * 0.03,
        "v_b": rng.standard_normal((E,), np.float32) * 0.03,
        "proj_w": rng.standard_normal((E, E), np.float32) * 0.03,
        "proj_b": rng.standard_normal((E,), np.float32) * 0.03,
        "ff_w": rng.standard_normal((E, E), np.float32) * 0.03,
        "ff_b": rng.standard_normal((E,), np.float32) * 0.03,
        "ln_g": np.ones((E,), np.float32),
        "ln_b": np.zeros((E,), np.float32),
    }
    o = kernel(**ins)
    print("ran", o.shape, o.dtype)

